# revision 1
# baseline (speedup 1.0000x reference)
"""Trainium2 Bass kernel for nn_EncoderLayer (B=4, N=2048, E=512, H=8, HIDDEN=1536).

Sharding: 8 cores; core c handles batch b=c//2, query-half c%2 (1024 query
rows). Each core computes K/V over the full 2048-row sequence of its batch
(keys are permutation-invariant under softmax, so the host rotates x[b] to put
the query rows first), and the FFN over its 1024 rows only.

Dataflow per core (all matmul operands bf16, accumulation fp32):
  LN1 token-major (bn_stats) -> PE-transpose xn -> xnT feature-major
  QKV: qT/kT feature-major, V token-major (+ones column -> softmax denom)
  scores S^T=[k,q] per head-pair (row-group concurrency), exp on ACT
  attnV accumulates [out^T | denom]; PE-transpose back; per-partition scale
  residual -> LN2 -> fc1 (evict raw, gelu deferred after all exps) -> fc2
"""

import sys

sys.path.insert(0, "/opt/trn_rl_repo")

import numpy as np
import ml_dtypes

B, N, E = 4, 2048, 512
H, HD = 8, 64
HID = 3 * E
NQ = 1024  # query rows per core
P = 128
EPS = 1e-5
NCORES = 8

_NC_CACHE = {}


def _build_nc(split_waits=True):
    from contextlib import ExitStack

    import concourse.bass as bass
    import concourse.mybir as mybir
    import concourse.tile as tile
    from concourse.masks import make_identity

    fp32 = mybir.dt.float32
    bf16 = mybir.dt.bfloat16
    AF = mybir.ActivationFunctionType
    ALU = mybir.AluOpType

    nc = bass.Bass()

    x_d = nc.declare_dram_parameter("x", [N, E], fp32, isOutput=False)
    wqkv_d = nc.declare_dram_parameter("wqkv", [E, 3 * E], bf16, isOutput=False)
    bqkv_d = nc.declare_dram_parameter("bqkv", [3 * E], fp32, isOutput=False)
    w1_d = nc.declare_dram_parameter("w1", [E, HID], bf16, isOutput=False)
    b1_d = nc.declare_dram_parameter("b1", [HID], fp32, isOutput=False)
    w2_d = nc.declare_dram_parameter("w2", [HID, E], bf16, isOutput=False)
    b2_d = nc.declare_dram_parameter("b2", [E], fp32, isOutput=False)
    out_d = nc.declare_dram_parameter("out", [NQ, E], fp32, isOutput=True)

    x_view = x_d[:].rearrange("(t p) e -> t p e", p=P)  # [16, 128, 512]
    out_view = out_d[:].rearrange("(t p) e -> t p e", p=P)  # [8, 128, 512]

    def bcast(ap, parts=P):
        return bass.AP(tensor=ap.tensor, offset=ap.offset, ap=[[0, parts]] + list(ap.ap))

    with tile.TileContext(nc) as tc, ExitStack() as ctx:
        const = ctx.enter_context(tc.tile_pool(name="const", bufs=1))
        big = ctx.enter_context(tc.tile_pool(name="big", bufs=1))
        wpool = ctx.enter_context(tc.tile_pool(name="wpool", bufs=2))
        work = ctx.enter_context(tc.tile_pool(name="work", bufs=3))
        expp = ctx.enter_context(tc.tile_pool(name="expp", bufs=6))
        psum = ctx.enter_context(tc.tile_pool(name="psum", bufs=2, space="PSUM"))

        id128 = const.tile([P, P], fp32)
        make_identity(nc, id128)
        id128b = const.tile([P, P], bf16)
        nc.vector.tensor_copy(out=id128b, in_=id128)
        eps_sb = const.tile([P, 1], fp32)
        nc.vector.memset(eps_sb, EPS)

        bq_sb = const.tile([P, 4], fp32)
        nc.sync.dma_start(out=bq_sb, in_=bqkv_d[:][0:512].rearrange("(c p) -> p c", p=P))
        bk_sb = const.tile([P, 4], fp32)
        nc.sync.dma_start(out=bk_sb, in_=bqkv_d[:][512:1024].rearrange("(c p) -> p c", p=P))
        bv_bc = const.tile([P, E], fp32)
        nc.sync.dma_start(out=bv_bc, in_=bcast(bqkv_d[:][1024:1536]))
        b1_sb = const.tile([P, 12], fp32)
        nc.sync.dma_start(out=b1_sb, in_=b1_d[:].rearrange("(c p) -> p c", p=P))
        b2_bc = const.tile([P, E], fp32)
        nc.sync.dma_start(out=b2_bc, in_=bcast(b2_d[:]))

        # wpool tag "w": two 16KB/partition slots rotating through
        # wqkv -> xnT -> w1 -> w2 (xnT is dead after phase B)
        wqkv_sb = wpool.tile([P, 4, 3 * E], bf16, tag="w")
        nc.sync.dma_start(out=wqkv_sb, in_=wqkv_d[:].rearrange("(c p) n -> p c n", p=P))
        xnT_sb = wpool.tile([P, 4, N], bf16, tag="w")  # LN1(x) feature-major

        xq_sb = big.tile([P, 8, E], fp32)       # raw x query rows; becomes x2 in place
        qT_sb = big.tile([P, 4, NQ], bf16)
        kT_sb = big.tile([P, 4, N], bf16)
        v_sb = big.tile([P, 16, H, HD + 1], bf16)  # token-major V + ones column
        att_sb = big.tile([P, 8, H, HD], bf16)
        xn2T_sb = big.tile([P, 4, NQ], bf16)
        g1T_sb = big.tile([P, 12, NQ], bf16)

        nc.vector.memset(v_sb[:, :, :, HD : HD + 1], 1.0)

        def layernorm_tile(xt, xn_out, apply_on_act=False):
            # rstd via exp(-0.5*ln(var+eps)): keeps ACT on the
            # natural_log_exp table set (shared with softmax exp) — no
            # table switching against the attention exp stream.
            stats = work.tile([P, 6], fp32, tag="st")
            nc.vector.bn_stats(out=stats, in_=xt)
            mv = work.tile([P, 2], fp32, tag="mv")
            nc.vector.bn_aggr(out=mv, in_=stats)
            lnv = work.tile([P, 1], fp32, tag="lnv")
            nc.scalar.activation(out=lnv, in_=mv[:, 1:2], func=AF.Ln, bias=eps_sb, scale=1.0)
            rstd = work.tile([P, 1], fp32, tag="rstd")
            nc.scalar.activation(out=rstd, in_=lnv, func=AF.Exp, scale=-0.5)
            # NOTE: ACT Identity with a per-partition scale AP crashes the
            # device (NRT_EXEC_UNIT_UNRECOVERABLE) — keep the apply on DVE.
            nc.vector.tensor_scalar(
                out=xn_out, in0=xt, scalar1=mv[:, 0:1], scalar2=rstd,
                op0=ALU.subtract, op1=ALU.mult,
            )

        def transpose_to(dstT, xn, tok):
            # 4 PE transposes (bf16, 1 cyc/row) of one [128tok, 512E] tile into
            # one psum bank, then a single strided DVE copy into
            # dstT[:, :, tok*128:(tok+1)*128]
            pt = psum.tile([P, 512], bf16, tag="tp")
            for ec in range(4):
                nc.tensor.transpose(
                    pt[:, ec * P : (ec + 1) * P], xn[:, ec * P : (ec + 1) * P], id128b
                )
            nc.vector.tensor_copy(
                out=dstT[:, :, tok * P : (tok + 1) * P],
                in_=pt.rearrange("p (c t) -> p c t", c=4),
            )

        # ---------------- Phase A: load x, LN1, transpose to xnT ----------------
        for t in range(16):
            if t < 8:
                xt = xq_sb[:, t, :]
            else:
                xt = work.tile([P, E], fp32, tag="xt")
            nc.sync.dma_start(out=xt, in_=x_view[t])
            xn = work.tile([P, E], bf16, tag="xn")
            layernorm_tile(xt, xn, apply_on_act=False)
            transpose_to(xnT_sb, xn, t)

        # ---------------- Phase B: QKV matmuls ----------------
        # per 512-token window so matmuls start as soon as that window's
        # transposes land (pipelines into phase A)
        for w in range(4):
            win = slice(w * 512, (w + 1) * 512)
            for m in range(4):  # kT channels m*128..
                pt = psum.tile([P, 512], fp32, tag="tp")
                for ec in range(4):
                    nc.tensor.matmul(
                        pt,
                        lhsT=wqkv_sb[:, ec, 512 + m * P : 512 + (m + 1) * P],
                        rhs=xnT_sb[:, ec, win],
                        start=(ec == 0), stop=(ec == 3),
                    )
                nc.vector.tensor_scalar_add(
                    out=kT_sb[:, m, win], in0=pt, scalar1=bk_sb[:, m : m + 1]
                )
            if w < 2:
                for m in range(4):  # qT channels
                    pt = psum.tile([P, 512], fp32, tag="tp")
                    for ec in range(4):
                        nc.tensor.matmul(
                            pt,
                            lhsT=wqkv_sb[:, ec, m * P : (m + 1) * P],
                            rhs=xnT_sb[:, ec, win],
                            start=(ec == 0), stop=(ec == 3),
                        )
                    nc.vector.tensor_scalar_add(
                        out=qT_sb[:, m, win], in0=pt, scalar1=bq_sb[:, m : m + 1]
                    )

        def v_block():
            # emitted after the first scores pair: V matmuls fill PE slack
            # under the first exp stream; only attnV needs them
            for tcn in range(16):
                pt = psum.tile([P, 512], fp32, tag="tp")
                for ec in range(4):
                    nc.tensor.matmul(
                        pt,
                        lhsT=xnT_sb[:, ec, tcn * P : (tcn + 1) * P],
                        rhs=wqkv_sb[:, ec, 1024:1536],
                        start=(ec == 0), stop=(ec == 3),
                    )
                nc.vector.tensor_copy(
                    out=v_sb[:, tcn, :, 0:HD],
                    in_=pt.rearrange("p (h d) -> p h d", h=H),
                )

        # ---------------- Phases C/D/E interleaved per query block ----------------
        w1_sb = wpool.tile([P, 4, HID], bf16, tag="w")
        nc.sync.dma_start(out=w1_sb, in_=w1_d[:].rearrange("(c p) n -> p c n", p=P))
        w2_sb = wpool.tile([P, 12, E], bf16, tag="w")
        nc.sync.dma_start(out=w2_sb, in_=w2_d[:].rearrange("(c p) n -> p c n", p=P))

        def scores_block(qb, jh):
            # halves[kh][:, kc, h2, :] = exp(scores/8) for head 2*jh+h2, keys
            # (kh*8+kc)*128..+128, bf16
            halves = []
            for kh in range(4):
                expSp = expp.tile([P, 4, 2, 512], bf16, tag="es")
                halves.append(expSp)
                for kc8 in range(4):
                    kc = kh * 4 + kc8
                    pt = psum.tile([P, 1024], fp32, tag="sc")
                    for h2 in range(2):
                        base = h2 * 64
                        nc.tensor.matmul(
                            pt[:, h2 * 512 : (h2 + 1) * 512],
                            lhsT=kT_sb[base : base + 64, jh, kc * P : (kc + 1) * P],
                            rhs=qT_sb[base : base + 64, jh, qb * 512 : (qb + 1) * 512],
                            start=True, stop=True,
                        )
                    nc.scalar.activation(
                        out=expSp[:, kc8, :, :], in_=pt, func=AF.Exp, scale=HD**-0.5
                    )
            return halves

        def attnv_block(qb, jh, halves):
            if True:  # keep indentation shallow
                for h2 in range(2):
                    h = 2 * jh + h2
                    pa = psum.tile([65, 512], fp32, tag="pa")
                    for kc in range(16):
                        nc.tensor.matmul(
                            pa,
                            lhsT=v_sb[:, kc, h, :],
                            rhs=halves[kc // 4][:, kc % 4, h2, :],
                            start=(kc == 0), stop=(kc == 15),
                        )
                    ah = work.tile([65, 512], fp32, tag="ah")
                    nc.vector.tensor_copy(out=ah, in_=pa)
                    pt2 = psum.tile([P, 4, 65], fp32, tag="tp")
                    for qs in range(4):
                        nc.tensor.transpose(
                            pt2[:, qs, :], ah[:, qs * P : (qs + 1) * P],
                            id128[0:65, 0:65],
                        )
                    for qs in range(4):
                        tcq = qb * 4 + qs
                        rec = work.tile([P, 1], fp32, tag="rec")
                        nc.vector.reciprocal(out=rec, in_=pt2[:, qs, 64:65])
                        nc.vector.tensor_scalar_mul(
                            out=att_sb[:, tcq, h, :], in0=pt2[:, qs, 0:HD], scalar1=rec
                        )

        def residual_ln2_block(qb):
            for tcn in range(qb * 4, qb * 4 + 4):
                x2t = xq_sb[:, tcn, :]
                nc.vector.tensor_tensor(
                    out=x2t, in0=x2t,
                    in1=att_sb[:, tcn].rearrange("p h d -> p (h d)"), op=ALU.add,
                )
                nc.vector.tensor_tensor(out=x2t, in0=x2t, in1=bv_bc, op=ALU.add)
                xn2 = work.tile([P, E], bf16, tag="xn")
                layernorm_tile(x2t, xn2, apply_on_act=False)
                transpose_to(xn2T_sb, xn2, tcn)
                # pre-add the fc2 bias into the residual now (LN2 already
                # consumed x2), shortening the final eviction to one add
                nc.vector.tensor_tensor(out=x2t, in0=x2t, in1=b2_bc, op=ALU.add)

        def fc1_block(qb, fuse_gelu):
            # fuse_gelu=False: raw bf16 evict, gelu deferred so ACT stays on
            # the exp table while attention is still running
            for mh in range(12):
                pt = psum.tile([P, 512], fp32, tag="tp")
                for ec in range(4):
                    nc.tensor.matmul(
                        pt,
                        lhsT=w1_sb[:, ec, mh * P : (mh + 1) * P],
                        rhs=xn2T_sb[:, ec, qb * 512 : (qb + 1) * 512],
                        start=(ec == 0), stop=(ec == 3),
                    )
                if fuse_gelu:
                    nc.scalar.activation(
                        out=g1T_sb[:, mh, qb * 512 : (qb + 1) * 512], in_=pt,
                        func=AF.Gelu, bias=b1_sb[:, mh : mh + 1], scale=1.0,
                    )
                else:
                    nc.vector.tensor_copy(
                        out=g1T_sb[:, mh, qb * 512 : (qb + 1) * 512], in_=pt
                    )

        h00 = scores_block(0, 0)
        v_block()
        attnv_block(0, 0, h00)
        for jh in range(1, 4):
            hh = scores_block(0, jh)
            attnv_block(0, jh, hh)
        residual_ln2_block(0)
        fc1_block(0, fuse_gelu=False)
        for jh in range(4):
            hh = scores_block(1, jh)
            attnv_block(1, jh, hh)
        residual_ln2_block(1)

        # all exps done; single table switch to gelu (in-place, fc1 bias) for
        # half 0; half 1's fc1 eviction IS the gelu; fc2 per half
        for mh in range(12):
            nc.scalar.activation(
                out=g1T_sb[:, mh, 0:512], in_=g1T_sb[:, mh, 0:512],
                func=AF.Gelu, bias=b1_sb[:, mh : mh + 1], scale=1.0,
            )
        fc1_block(1, fuse_gelu=True)
        for qb in range(2):
            for tcn in range(qb * 4, qb * 4 + 4):
                pt = psum.tile([P, 512], fp32, tag="tp")
                for j in range(12):
                    nc.tensor.matmul(
                        pt,
                        lhsT=g1T_sb[:, j, tcn * P : (tcn + 1) * P],
                        rhs=w2_sb[:, j, :],
                        start=(j == 0), stop=(j == 11),
                    )
                ot = work.tile([P, E], fp32, tag="ot")
                nc.vector.tensor_tensor(out=ot, in0=pt, in1=xq_sb[:, tcn, :], op=ALU.add)
                nc.sync.dma_start(out=out_view[tcn], in_=ot)

    if split_waits:
        _split_matmul_waits(nc, mybir)
    return nc


def _split_matmul_waits(nc, mybir):
    """walrus allows only one sync wait per engine instruction; hoist extra
    waits onto same-engine NoOps placed just before (NX dispatch is in-order,
    so the nops' waits gate the instruction)."""
    k = 0
    for fn in nc.m.functions:
        for blk in fn.blocks:
            new = []
            for inst in blk.instructions:
                si = inst.sync_info
                if si is not None and si.on_wait and len(si.on_wait) > 1:
                    for w in si.on_wait[:-1]:
                        nop = mybir.InstNoOp(name=f"waitnop-{k}", ins=[], outs=[])
                        k += 1
                        nop.engine = inst.engine
                        nop.sync_info = mybir.SyncInfo(on_wait=[w], on_update=[])
                        new.append(nop)
                    inst.sync_info = mybir.SyncInfo(
                        on_wait=[si.on_wait[-1]], on_update=si.on_update
                    )
                new.append(inst)
            blk.instructions[:] = new


def _get_nc():
    if "nc" not in _NC_CACHE:
        _NC_CACHE["nc"] = _build_nc()
    return _NC_CACHE["nc"]


def _prep_inputs(inputs):
    x = np.asarray(inputs["x"], np.float32)
    qkv_w = np.asarray(inputs["qkv_w"], np.float32)
    qkv_b = np.asarray(inputs["qkv_b"], np.float32)
    fc1_w = np.asarray(inputs["fc1_w"], np.float32)
    fc1_b = np.asarray(inputs["fc1_b"], np.float32)
    fc2_w = np.asarray(inputs["fc2_w"], np.float32)
    fc2_b = np.asarray(inputs["fc2_b"], np.float32)

    # reorder qkv channels: per-head interleave [q|k|v]*H -> heads-major [Q|K|V]
    w3 = qkv_w.reshape(E, H, 3, HD)
    wqkv = np.ascontiguousarray(
        np.concatenate([w3[:, :, i, :].reshape(E, E) for i in range(3)], axis=1)
    ).astype(ml_dtypes.bfloat16)
    b3 = qkv_b.reshape(H, 3, HD)
    bqkv = np.ascontiguousarray(
        np.concatenate([b3[:, i, :].reshape(E) for i in range(3)], axis=0)
    )

    w1 = np.ascontiguousarray(fc1_w).astype(ml_dtypes.bfloat16)
    w2 = np.ascontiguousarray(fc2_w).astype(ml_dtypes.bfloat16)

    in_maps = []
    for c in range(NCORES):
        b, half = c // 2, c % 2
        xr = np.ascontiguousarray(np.roll(x[b], -half * NQ, axis=0))
        in_maps.append(
            {
                "x": xr,
                "wqkv": wqkv,
                "bqkv": bqkv,
                "w1": w1,
                "b1": fc1_b,
                "w2": w2,
                "b2": fc2_b,
            }
        )
    return in_maps


def kernel(**inputs) -> np.ndarray:
    from concourse.bass_utils import run_bass_kernel_spmd

    nc = _get_nc()
    in_maps = _prep_inputs(inputs)
    res = run_bass_kernel_spmd(nc, in_maps, core_ids=list(range(NCORES)))
    y = np.empty((B, N, E), np.float32)
    for c in range(NCORES):
        b, half = c // 2, c % 2
        y[b, half * NQ : (half + 1) * NQ] = np.asarray(res.results[c]["out"])
    return y


if __name__ == "__main__":
    nc = _build_nc()
    print("build OK")



# revision 19
# speedup vs baseline: 1.2116x; 1.2116x over previous
"""Trainium2 Bass kernel for nn_EncoderLayer (B=4, N=2048, E=512, H=8, HIDDEN=1536).

Sharding: 8 cores; core c handles batch b=c//2, query-half c%2 (1024 query
rows). Each core computes K/V over the full 2048-row sequence of its batch
(keys are permutation-invariant under softmax, so the host rotates x[b] to put
the query rows first), and the FFN over its 1024 rows only.

All big matmuls run in fp8e4m3 DoubleRow perf mode (2 reduction rows per
partition, 0.5 cyc/row): QKV / scores / attnV / fc1 / fc2. The residual path,
LN statistics, softmax scores (PSUM) and denominators stay fp32; transposes
ride bf16. Channel layouts are slot-paired for DoubleRow:
  qkv/fc1/fc2 contraction c -> (pair ecp = c//256, slot s = (c//128)%2, p = c%128)
  per-head qk contraction ch -> (partition 32*g + ch%32, slot ch//32), head
  quartets a=(0..3)/b=(4..7) stacked along partitions (host permutes wqkv
  q/k columns so PSUM evictions land partition-aligned).

Engine split: PE matmuls/transposes; ACT exp spine (softmax) + deferred gelu
(one table switch) + LN rstd via Ln/Exp; DVE LN stats/apply, q/k evictions,
softmax scale; Pool (otherwise idle) takes psum->sbuf copies (v8, xnT, fc1
raw) and residual adds.
"""

import sys

sys.path.insert(0, "/opt/trn_rl_repo")

import numpy as np
import ml_dtypes

B, N, E = 4, 2048, 512
H, HD = 8, 64
HID = 3 * E
NQ = 1024  # query rows per core
P = 128
EPS = 1e-5
NCORES = 8

_NC_CACHE = {}


def _build_nc(split_waits=True):
    from contextlib import ExitStack

    import concourse.bass as bass
    import concourse.mybir as mybir
    import concourse.tile as tile
    from concourse.masks import make_identity

    fp32 = mybir.dt.float32
    bf16 = mybir.dt.bfloat16
    fp8 = mybir.dt.float8e4
    AF = mybir.ActivationFunctionType
    ALU = mybir.AluOpType
    DR = mybir.MatmulPerfMode.DoubleRow

    nc = bass.Bass()

    x_d = nc.declare_dram_parameter("x", [N, E], fp32, isOutput=False)
    wqkv_d = nc.declare_dram_parameter("wqkv", [E, 3 * E], fp8, isOutput=False)
    bq_d = nc.declare_dram_parameter("bq", [P, 4], fp32, isOutput=False)
    bk_d = nc.declare_dram_parameter("bk", [P, 4], fp32, isOutput=False)
    bv_d = nc.declare_dram_parameter("bv", [E], fp32, isOutput=False)
    w1_d = nc.declare_dram_parameter("w1", [E, HID], fp8, isOutput=False)
    b1_d = nc.declare_dram_parameter("b1", [HID], fp32, isOutput=False)
    w2_d = nc.declare_dram_parameter("w2", [HID, E], fp8, isOutput=False)
    b2_d = nc.declare_dram_parameter("b2", [E], fp32, isOutput=False)
    out_d = nc.declare_dram_parameter("out", [NQ, E], fp32, isOutput=True)

    x_view = x_d[:].rearrange("(t p) e -> t p e", p=P)  # [16, 128, 512]
    out_view = out_d[:].rearrange("(t p) e -> t p e", p=P)  # [8, 128, 512]

    def bcast(ap, parts=P):
        return bass.AP(tensor=ap.tensor, offset=ap.offset, ap=[[0, parts]] + list(ap.ap))

    with tile.TileContext(nc) as tc, ExitStack() as ctx:
        const = ctx.enter_context(tc.tile_pool(name="const", bufs=1))
        big = ctx.enter_context(tc.tile_pool(name="big", bufs=1))
        wpool = ctx.enter_context(tc.tile_pool(name="wpool", bufs=2))
        work = ctx.enter_context(tc.tile_pool(name="work", bufs=3))
        expp = ctx.enter_context(tc.tile_pool(name="expp", bufs=6))
        psum = ctx.enter_context(tc.tile_pool(name="psum", bufs=2, space="PSUM"))
        psum1 = ctx.enter_context(tc.tile_pool(name="psum1", bufs=1, space="PSUM"))

        id128 = const.tile([P, P], fp32)
        make_identity(nc, id128)
        id128b = const.tile([P, P], bf16)
        nc.vector.tensor_copy(out=id128b, in_=id128)
        eps_sb = const.tile([P, 1], fp32)
        nc.vector.memset(eps_sb, EPS)
        # DoubleRow ones for the softmax denominator (DR psum outputs must
        # start at partition 0, so the denominator rides spare pa columns)
        ones8 = const.tile([P, 2, 32], fp8)
        nc.vector.memset(ones8, 1.0)

        bq_sb = const.tile([P, 4], fp32)
        nc.sync.dma_start(out=bq_sb, in_=bq_d[:])
        bk_sb = const.tile([P, 4], fp32)
        nc.sync.dma_start(out=bk_sb, in_=bk_d[:])
        bv_bc = const.tile([P, E], fp32)
        nc.sync.dma_start(out=bv_bc, in_=bcast(bv_d[:]))
        b1_sb = const.tile([P, 12], fp32)
        nc.sync.dma_start(out=b1_sb, in_=b1_d[:].rearrange("(c p) -> p c", p=P))
        b2_bc = const.tile([P, E], fp32)
        nc.sync.dma_start(out=b2_bc, in_=bcast(b2_d[:]))

        # wpool tag "w": two 8KB/partition slots rotating through
        # wqkv -> xnT -> w1 -> w2 (xnT is dead after phase B)
        wqkv_sb = wpool.tile([P, 4, 3 * E], fp8, tag="w")
        nc.sync.dma_start(out=wqkv_sb, in_=wqkv_d[:].rearrange("(c p) n -> p c n", p=P))
        xnT_sb = wpool.tile([P, 4, N], fp8, tag="w")  # LN1(x) feature-major

        xq_sb = big.tile([P, 8, E], fp32)       # raw x query rows; becomes x2 in place
        qT_a = big.tile([P, 2, NQ], fp8)        # heads 0-3, partition 32g+c, slot s
        qT_b = big.tile([P, 2, NQ], fp8)        # heads 4-7
        kT_a = big.tile([P, 2, N], fp8)
        kT_b = big.tile([P, 2, N], fp8)
        v_sb = big.tile([P, 16, H, HD], fp8)    # token-major V
        att_sb = big.tile([P, 8, H, HD], bf16)
        xn2T_sb = big.tile([P, 4, NQ], fp8)
        g1T_sb = big.tile([P, 12, NQ], fp8)

        def layernorm_tile(xt, xn_out):
            # rstd via exp(-0.5*ln(var+eps)): keeps ACT on the
            # natural_log_exp table set (shared with softmax exp) — no
            # table switching against the attention exp stream.
            stats = work.tile([P, 6], fp32, tag="st")
            nc.vector.bn_stats(out=stats, in_=xt)
            mv = work.tile([P, 2], fp32, tag="mv")
            nc.vector.bn_aggr(out=mv, in_=stats)
            lnv = work.tile([P, 1], fp32, tag="lnv")
            nc.scalar.activation(out=lnv, in_=mv[:, 1:2], func=AF.Ln, bias=eps_sb, scale=1.0)
            rstd = work.tile([P, 1], fp32, tag="rstd")
            nc.scalar.activation(out=rstd, in_=lnv, func=AF.Exp, scale=-0.5)
            # NOTE: ACT Identity with a per-partition scale AP crashes the
            # device (NRT_EXEC_UNIT_UNRECOVERABLE). The apply runs on Pool
            # (SBUF-only op, Pool is otherwise idle) to keep DVE free.
            nc.gpsimd.tensor_scalar(
                out=xn_out, in0=xt, scalar1=mv[:, 0:1], scalar2=rstd,
                op0=ALU.subtract, op1=ALU.mult,
            )

        def transpose_to(dstT, xn, tok):
            # 4 PE transposes (bf16, 1 cyc/row) of one [128tok, 512E] tile into
            # one psum bank, then a single strided DVE copy (fp8 out; GPSIMD
            # cannot read PSUM) into dstT[:, :, tok*128:(tok+1)*128]
            pt = psum.tile([P, 512], bf16, tag="tp")
            for ec in range(4):
                nc.tensor.transpose(
                    pt[:, ec * P : (ec + 1) * P], xn[:, ec * P : (ec + 1) * P], id128b
                )
            nc.vector.tensor_copy(
                out=dstT[:, :, tok * P : (tok + 1) * P],
                in_=pt.rearrange("p (c t) -> p c t", c=4),
            )

        # ---------------- Phase A: load x, LN1, transpose to xnT ----------------
        for t in range(16):
            if t < 8:
                xt = xq_sb[:, t, :]
            else:
                xt = work.tile([P, E], fp32, tag="xt")
            nc.sync.dma_start(out=xt, in_=x_view[t])
            xn = work.tile([P, E], bf16, tag="xn")
            layernorm_tile(xt, xn)
            transpose_to(xnT_sb, xn, t)

        # ---------------- Phase B: QKV matmuls (fp8 DoubleRow) ----------------
        # per 512-token window so matmuls start as soon as that window's
        # transposes land (pipelines into phase A)
        kT_x = [kT_a, kT_b]
        qT_x = [qT_a, qT_b]
        for w in range(4):
            win = slice(w * 512, (w + 1) * 512)
            for m in range(4):  # (quartet x, slot s) = (m//2, m%2)
                pt = psum.tile([P, 512], fp32, tag="tp")
                for ecp in range(2):
                    nc.tensor.matmul(
                        pt,
                        lhsT=wqkv_sb[:, 2 * ecp : 2 * ecp + 2, 512 + m * P : 512 + (m + 1) * P],
                        rhs=xnT_sb[:, 2 * ecp : 2 * ecp + 2, win],
                        start=(ecp == 0), stop=(ecp == 1), perf_mode=DR,
                    )
                nc.vector.tensor_scalar_add(
                    out=kT_x[m // 2][:, m % 2, win], in0=pt, scalar1=bk_sb[:, m : m + 1]
                )
            if w < 2:
                for m in range(4):
                    pt = psum.tile([P, 512], fp32, tag="tp")
                    for ecp in range(2):
                        nc.tensor.matmul(
                            pt,
                            lhsT=wqkv_sb[:, 2 * ecp : 2 * ecp + 2, m * P : (m + 1) * P],
                            rhs=xnT_sb[:, 2 * ecp : 2 * ecp + 2, win],
                            start=(ecp == 0), stop=(ecp == 1), perf_mode=DR,
                        )
                    nc.vector.tensor_scalar_add(
                        out=qT_x[m // 2][:, m % 2, win], in0=pt, scalar1=bq_sb[:, m : m + 1]
                    )

        def v_block():
            # emitted after the first scores pair: V matmuls fill PE slack
            # under the first exp stream; only attnV needs them
            for tcn in range(16):
                pt = psum.tile([P, 512], fp32, tag="tp")
                for ecp in range(2):
                    nc.tensor.matmul(
                        pt,
                        lhsT=xnT_sb[:, 2 * ecp : 2 * ecp + 2, tcn * P : (tcn + 1) * P],
                        rhs=wqkv_sb[:, 2 * ecp : 2 * ecp + 2, 1024:1536],
                        start=(ecp == 0), stop=(ecp == 1), perf_mode=DR,
                    )
                nc.vector.tensor_copy(
                    out=v_sb[:, tcn, :, :],
                    in_=pt.rearrange("p (h d) -> p h d", h=H),
                )

        # ---------------- Phases C/D/E interleaved per query block ----------------
        w1_sb = wpool.tile([P, 4, HID], fp8, tag="w")
        nc.sync.dma_start(out=w1_sb, in_=w1_d[:].rearrange("(c p) n -> p c n", p=P))
        w2_sb = wpool.tile([P, 12, E], fp8, tag="w")
        nc.sync.dma_start(out=w2_sb, in_=w2_d[:].rearrange("(c p) n -> p c n", p=P))

        def scores_block(qb, jh):
            # halves[kh][:, kc8, h2, :] = exp(scores/8) fp8 for head 2*jh+h2,
            # keys (kh*4+kc8)*128..+128
            kt, qt = kT_x[jh // 2], qT_x[jh // 2]
            g0 = 2 * (jh % 2)
            halves = []
            for kh in range(4):
                expSp = expp.tile([P, 4, 2, 512], fp8, tag="es")
                halves.append(expSp)
                for kc8 in range(4):
                    kc = kh * 4 + kc8
                    pt = psum.tile([P, 1024], fp32, tag="sc")
                    for h2 in range(2):
                        g = g0 + h2
                        nc.tensor.matmul(
                            pt[:, h2 * 512 : (h2 + 1) * 512],
                            lhsT=kt[32 * g : 32 * g + 32, :, kc * P : (kc + 1) * P],
                            rhs=qt[32 * g : 32 * g + 32, :, qb * 512 : (qb + 1) * 512],
                            start=True, stop=True, perf_mode=DR,
                            tile_position=(32 * g, 0),
                        )
                    nc.scalar.activation(
                        out=expSp[:, kc8, :, :], in_=pt, func=AF.Exp, scale=HD**-0.5
                    )
            return halves

        def attnv_block(qb, jh, halves):
            for h2 in range(2):
                h = 2 * jh + h2
                # attnV out in cols 0:512, softmax denominator (32 identical
                # rows from the ones stationary) in cols 512:1024
                pa = psum1.tile([64, 1024], fp32, tag="pa")
                for kc2 in range(8):
                    kc = 2 * kc2
                    nc.tensor.matmul(
                        pa[:, 0:512],
                        lhsT=v_sb[:, kc : kc + 2, h, :],
                        rhs=halves[kc // 4][:, kc % 4 : kc % 4 + 2, h2, :],
                        start=(kc2 == 0), stop=(kc2 == 7), perf_mode=DR,
                    )
                for kc2 in range(8):
                    kc = 2 * kc2
                    nc.tensor.matmul(
                        pa[0:32, 512:1024],
                        lhsT=ones8,
                        rhs=halves[kc // 4][:, kc % 4 : kc % 4 + 2, h2, :],
                        start=(kc2 == 0), stop=(kc2 == 7), perf_mode=DR,
                        skip_group_check=True,
                    )
                ah = work.tile([65, 512], fp32, tag="ah")
                nc.vector.tensor_copy(out=ah[0:64, :], in_=pa[:, 0:512])
                nc.vector.tensor_copy(out=ah[64:65, :], in_=pa[0:1, 512:1024])
                pt2 = psum.tile([P, 4, 65], fp32, tag="tp")
                for qs in range(4):
                    nc.tensor.transpose(
                        pt2[:, qs, :], ah[:, qs * P : (qs + 1) * P],
                        id128[0:65, 0:65],
                    )
                for qs in range(4):
                    tcq = qb * 4 + qs
                    rec = work.tile([P, 1], fp32, tag="rec")
                    nc.vector.reciprocal(out=rec, in_=pt2[:, qs, 64:65])
                    nc.vector.tensor_scalar_mul(
                        out=att_sb[:, tcq, h, :], in0=pt2[:, qs, 0:HD], scalar1=rec
                    )

        def residual_ln2_block(qb):
            for tcn in range(qb * 4, qb * 4 + 4):
                x2t = xq_sb[:, tcn, :]
                nc.gpsimd.tensor_tensor(
                    out=x2t, in0=x2t,
                    in1=att_sb[:, tcn].rearrange("p h d -> p (h d)"), op=ALU.add,
                )
                nc.gpsimd.tensor_tensor(out=x2t, in0=x2t, in1=bv_bc, op=ALU.add)
                xn2 = work.tile([P, E], bf16, tag="xn")
                layernorm_tile(x2t, xn2)
                transpose_to(xn2T_sb, xn2, tcn)
                # pre-add the fc2 bias into the residual now (LN2 already
                # consumed x2), shortening the final eviction to one add
                nc.gpsimd.tensor_tensor(out=x2t, in0=x2t, in1=b2_bc, op=ALU.add)

        def fc1_block(qb, fuse_gelu):
            # fuse_gelu=False: raw fp8 evict, gelu deferred so ACT stays on
            # the exp table while attention is still running
            for mh in range(12):
                pt = psum.tile([P, 512], fp32, tag="tp")
                for ecp in range(2):
                    nc.tensor.matmul(
                        pt,
                        lhsT=w1_sb[:, 2 * ecp : 2 * ecp + 2, mh * P : (mh + 1) * P],
                        rhs=xn2T_sb[:, 2 * ecp : 2 * ecp + 2, qb * 512 : (qb + 1) * 512],
                        start=(ecp == 0), stop=(ecp == 1), perf_mode=DR,
                    )
                if fuse_gelu:
                    nc.scalar.activation(
                        out=g1T_sb[:, mh, qb * 512 : (qb + 1) * 512], in_=pt,
                        func=AF.Gelu, bias=b1_sb[:, mh : mh + 1], scale=1.0,
                    )
                else:
                    nc.vector.tensor_copy(
                        out=g1T_sb[:, mh, qb * 512 : (qb + 1) * 512], in_=pt
                    )

        h00 = scores_block(0, 0)
        v_block()
        attnv_block(0, 0, h00)
        for jh in range(1, 4):
            hh = scores_block(0, jh)
            attnv_block(0, jh, hh)
        residual_ln2_block(0)
        fc1_block(0, fuse_gelu=False)
        for jh in range(4):
            hh = scores_block(1, jh)
            attnv_block(1, jh, hh)
        residual_ln2_block(1)

        # all exps done; single table switch to gelu (in-place, fc1 bias) for
        # half 0; half 1's fc1 eviction IS the gelu; fc2 per half
        for mh in range(12):
            nc.scalar.activation(
                out=g1T_sb[:, mh, 0:512], in_=g1T_sb[:, mh, 0:512],
                func=AF.Gelu, bias=b1_sb[:, mh : mh + 1], scale=1.0,
            )
        fc1_block(1, fuse_gelu=True)
        for qb in range(2):
            for tcn in range(qb * 4, qb * 4 + 4):
                pt = psum.tile([P, 512], fp32, tag="tp")
                for jp in range(6):
                    nc.tensor.matmul(
                        pt,
                        lhsT=g1T_sb[:, 2 * jp : 2 * jp + 2, tcn * P : (tcn + 1) * P],
                        rhs=w2_sb[:, 2 * jp : 2 * jp + 2, :],
                        start=(jp == 0), stop=(jp == 5), perf_mode=DR,
                    )
                ot = work.tile([P, E], fp32, tag="ot")
                nc.vector.tensor_tensor(out=ot, in0=pt, in1=xq_sb[:, tcn, :], op=ALU.add)
                nc.sync.dma_start(out=out_view[tcn], in_=ot)

    if split_waits:
        _split_matmul_waits(nc, mybir)
    return nc


def _split_matmul_waits(nc, mybir):
    """walrus allows only one sync wait per engine instruction; hoist extra
    waits onto same-engine NoOps placed just before (NX dispatch is in-order,
    so the nops' waits gate the instruction)."""
    k = 0
    for fn in nc.m.functions:
        for blk in fn.blocks:
            new = []
            for inst in blk.instructions:
                si = inst.sync_info
                if si is not None and si.on_wait and len(si.on_wait) > 1:
                    for w in si.on_wait[:-1]:
                        nop = mybir.InstNoOp(name=f"waitnop-{k}", ins=[], outs=[])
                        k += 1
                        nop.engine = inst.engine
                        nop.sync_info = mybir.SyncInfo(on_wait=[w], on_update=[])
                        new.append(nop)
                    inst.sync_info = mybir.SyncInfo(
                        on_wait=[si.on_wait[-1]], on_update=si.on_update
                    )
                new.append(inst)
            blk.instructions[:] = new


def _get_nc():
    if "nc" not in _NC_CACHE:
        _NC_CACHE["nc"] = _build_nc()
    return _NC_CACHE["nc"]


def _qk_slot_perm():
    # perm[j]: source channel (within a heads-major 512-col q or k section)
    # for permuted column j = m*128 + p, m = (quartet x)*2 + slot s; the
    # psum partition p then lands head 4x + p//32, channel 32s + p%32.
    perm = np.empty(512, np.int64)
    for m in range(4):
        x, s = m // 2, m % 2
        for p in range(P):
            perm[m * 128 + p] = (4 * x + p // 32) * 64 + s * 32 + (p % 32)
    return perm


def _prep_inputs(inputs):
    fp8 = ml_dtypes.float8_e4m3
    x = np.asarray(inputs["x"], np.float32)
    qkv_w = np.asarray(inputs["qkv_w"], np.float32)
    qkv_b = np.asarray(inputs["qkv_b"], np.float32)
    fc1_w = np.asarray(inputs["fc1_w"], np.float32)
    fc1_b = np.asarray(inputs["fc1_b"], np.float32)
    fc2_w = np.asarray(inputs["fc2_w"], np.float32)
    fc2_b = np.asarray(inputs["fc2_b"], np.float32)

    # reorder qkv channels: per-head interleave [q|k|v]*H -> heads-major
    # [Q|K|V], then slot-permute Q and K columns for the DoubleRow layout
    w3 = qkv_w.reshape(E, H, 3, HD)
    wq, wk, wv = (w3[:, :, i, :].reshape(E, E) for i in range(3))
    b3 = qkv_b.reshape(H, 3, HD)
    bq, bk, bv = (np.ascontiguousarray(b3[:, i, :].reshape(E)) for i in range(3))

    perm = _qk_slot_perm()
    wqkv = np.ascontiguousarray(
        np.concatenate([wq[:, perm], wk[:, perm], wv], axis=1)
    ).astype(fp8)
    bq_t = np.ascontiguousarray(bq[perm].reshape(4, P).T)  # [P, 4]
    bk_t = np.ascontiguousarray(bk[perm].reshape(4, P).T)

    w1 = np.ascontiguousarray(fc1_w).astype(fp8)
    w2 = np.ascontiguousarray(fc2_w).astype(fp8)

    in_maps = []
    for c in range(NCORES):
        b, half = c // 2, c % 2
        xr = np.ascontiguousarray(np.roll(x[b], -half * NQ, axis=0))
        in_maps.append(
            {
                "x": xr,
                "wqkv": wqkv,
                "bq": bq_t,
                "bk": bk_t,
                "bv": bv,
                "w1": w1,
                "b1": fc1_b,
                "w2": w2,
                "b2": fc2_b,
            }
        )
    return in_maps


def kernel(**inputs) -> np.ndarray:
    from concourse.bass_utils import run_bass_kernel_spmd

    nc = _get_nc()
    in_maps = _prep_inputs(inputs)
    res = run_bass_kernel_spmd(nc, in_maps, core_ids=list(range(NCORES)))
    y = np.empty((B, N, E), np.float32)
    for c in range(NCORES):
        b, half = c // 2, c % 2
        y[b, half * NQ : (half + 1) * NQ] = np.asarray(res.results[c]["out"])
    return y


if __name__ == "__main__":
    nc = _build_nc()
    print("build OK")


# revision 47
# speedup vs baseline: 1.2704x; 1.0485x over previous
"""Trainium2 Bass kernel for nn_EncoderLayer (B=4, N=2048, E=512, H=8, HIDDEN=1536).

Sharding: 8 cores; core c handles batch b=c//2, query-half c%2 (1024 query
rows). Each core computes K/V over the full 2048-row sequence of its batch
(keys are permutation-invariant under softmax, so the host rotates x[b] to put
the query rows first), and the FFN over its 1024 rows only.

Precision (driven by an error-attribution study against the fp32 reference):
attention (q/k/v, exp(scores), attnV) and fc2 run in fp8e4m3 DoubleRow perf
mode (2 reduction rows/partition, 0.5 cyc/row) — their quantization noise
washes out against the residual path. QKV projection and fc1 stay bf16 (fc1
error feeds gelu+fc2 directly and dominated the budget). Residual/LN fp32.

Layouts: per-head qk contraction ch -> (partition 32*g + ch%32, slot ch//32),
head quartets a=(0..3)/b=(4..7) stacked along partitions (host permutes wqkv
q/k columns so PSUM evictions land partition-aligned). fc2 contraction
hc -> (pair jp = hc//256, slot (hc//128)%2, p = hc%128) falls out of the
[P, 12, NQ] g1 layout for free.

Engine split: PE matmuls + attention-out transposes; ACT exp spine + gelu
(deferred/fused to bracket one table switch) + LN rstd via Ln/Exp (exp-table
resident); DVE LN stats, psum evictions, softmax scale; Pool (idle otherwise)
LN applies and residual adds; DMA engines do the LN1 transposes
(dma_start_transpose), keeping DVE off the phase-A critical path.
"""

import sys

sys.path.insert(0, "/opt/trn_rl_repo")

import numpy as np
import ml_dtypes

B, N, E = 4, 2048, 512
H, HD = 8, 64
HID = 3 * E
NQ = 1024  # query rows per core
P = 128
EPS = 1e-5
NCORES = 8

_NC_CACHE = {}


def _build_nc(split_waits=True):
    from contextlib import ExitStack

    import concourse.bass as bass
    import concourse.mybir as mybir
    import concourse.tile as tile
    from concourse.masks import make_identity

    fp32 = mybir.dt.float32
    bf16 = mybir.dt.bfloat16
    fp8 = mybir.dt.float8e4
    AF = mybir.ActivationFunctionType
    ALU = mybir.AluOpType
    DR = mybir.MatmulPerfMode.DoubleRow

    nc = bass.Bass()

    x_d = nc.declare_dram_parameter("x", [N, E], fp32, isOutput=False)
    wqkv_d = nc.declare_dram_parameter("wqkv", [E, 3 * E], bf16, isOutput=False)
    bq_d = nc.declare_dram_parameter("bq", [P, 4], fp32, isOutput=False)
    bk_d = nc.declare_dram_parameter("bk", [P, 4], fp32, isOutput=False)
    bv_d = nc.declare_dram_parameter("bv", [E], fp32, isOutput=False)
    w1_d = nc.declare_dram_parameter("w1", [E, HID], bf16, isOutput=False)
    b1_d = nc.declare_dram_parameter("b1", [HID], fp32, isOutput=False)
    w2_d = nc.declare_dram_parameter("w2", [HID, E], fp8, isOutput=False)
    b2_d = nc.declare_dram_parameter("b2", [E], fp32, isOutput=False)
    out_d = nc.declare_dram_parameter("out", [NQ, E], fp32, isOutput=True)

    x_view = x_d[:].rearrange("(t p) e -> t p e", p=P)  # [16, 128, 512]
    out_view = out_d[:].rearrange("(t p) e -> t p e", p=P)  # [8, 128, 512]

    def bcast(ap, parts=P):
        return bass.AP(tensor=ap.tensor, offset=ap.offset, ap=[[0, parts]] + list(ap.ap))

    with tile.TileContext(nc) as tc, ExitStack() as ctx:
        const = ctx.enter_context(tc.tile_pool(name="const", bufs=1))
        big = ctx.enter_context(tc.tile_pool(name="big", bufs=1))
        wpool = ctx.enter_context(tc.tile_pool(name="wpool", bufs=2))
        work = ctx.enter_context(tc.tile_pool(name="work", bufs=3))
        xtp = ctx.enter_context(tc.tile_pool(name="xtp", bufs=4))
        expp = ctx.enter_context(tc.tile_pool(name="expp", bufs=12))
        psum = ctx.enter_context(tc.tile_pool(name="psum", bufs=2, space="PSUM"))
        psum1 = ctx.enter_context(tc.tile_pool(name="psum1", bufs=1, space="PSUM"))

        id128 = const.tile([P, P], fp32)
        make_identity(nc, id128)
        id128b = const.tile([P, P], bf16)
        nc.vector.tensor_copy(out=id128b, in_=id128)
        eps_sb = const.tile([P, 1], fp32)
        nc.vector.memset(eps_sb, EPS)
        # DoubleRow ones for the softmax denominator (DR psum outputs must
        # start at partition 0, so the denominator rides spare pa columns)
        ones8 = const.tile([P, 2, 32], fp8)
        nc.vector.memset(ones8, 1.0)

        bq_sb = const.tile([P, 4], fp32)
        nc.sync.dma_start(out=bq_sb, in_=bq_d[:])
        bk_sb = const.tile([P, 4], fp32)
        nc.sync.dma_start(out=bk_sb, in_=bk_d[:])
        bv_bc = const.tile([P, E], fp32)
        nc.sync.dma_start(out=bv_bc, in_=bcast(bv_d[:]))
        b1_sb = const.tile([P, 12], fp32)
        nc.sync.dma_start(out=b1_sb, in_=b1_d[:].rearrange("(c p) -> p c", p=P))
        b2_bc = const.tile([P, E], fp32)
        nc.sync.dma_start(out=b2_bc, in_=bcast(b2_d[:]))

        # wpool tag "w": two slots rotating through wqkv -> xnT -> w1 -> w2
        # (xnT is dead after phase B+V)
        wqkv_sb = wpool.tile([P, 4, 3 * E], bf16, tag="w")
        nc.sync.dma_start(out=wqkv_sb, in_=wqkv_d[:].rearrange("(c p) n -> p c n", p=P))
        # LN1(x) transposed, token-block-major so each dma_start_transpose
        # writes a contiguous [P, 4, 128] destination
        xnT_sb = wpool.tile([P, 16, 4, P], bf16, tag="w")

        xq_sb = big.tile([P, 8, E], fp32)       # raw x query rows; becomes x2 in place
        qT_a = big.tile([P, 2, NQ], fp8)        # heads 0-3, partition 32g+c, slot s
        qT_b = big.tile([P, 2, NQ], fp8)        # heads 4-7
        kT_a = big.tile([P, 2, N], fp8)
        kT_b = big.tile([P, 2, N], fp8)
        v_sb = big.tile([P, 16, H, HD], fp8)    # token-major V
        att_sb = big.tile([P, 8, H, HD], bf16)
        xn2T_sb = big.tile([P, 4, NQ], bf16)
        g1r_sb = big.tile([P, 12, 512], bf16)   # fc1(qb=0) raw (bias added)
        g1T_sb = big.tile([P, 12, NQ], fp8)

        def ln_group(xts, xn_outs, batched=True):
            # LN over up to 4 token tiles; batched=True shares one Ln+Exp
            # pair across the group (fewer ACT instrs, +latency), while
            # batched=False pipelines per tile (for the latency-critical
            # tail). rstd via exp(-0.5*ln(var+eps)) keeps ACT on the
            # natural_log_exp table set (shared with softmax exp) — no
            # table switching against the attention exp stream.
            n = len(xts)
            groups = [range(n)] if batched else [[i] for i in range(n)]
            mv = work.tile([P, n, 2], fp32, tag="mv")
            for idxs in groups:
                for i in idxs:
                    stats = work.tile([P, 6], fp32, tag="st")
                    nc.vector.bn_stats(out=stats, in_=xts[i])
                    nc.vector.bn_aggr(out=mv[:, i, :], in_=stats)
                i0, ng = idxs[0], len(idxs)
                lnv = work.tile([P, ng], fp32, tag="lnv")
                nc.scalar.activation(
                    out=lnv, in_=mv[:, i0 : i0 + ng, 1], func=AF.Ln, bias=eps_sb, scale=1.0
                )
                rstd = work.tile([P, ng], fp32, tag="rstd")
                nc.scalar.activation(out=rstd, in_=lnv, func=AF.Exp, scale=-0.5)
                # NOTE: ACT Identity with a per-partition scale AP crashes the
                # device (NRT_EXEC_UNIT_UNRECOVERABLE). The apply runs on Pool
                # (SBUF-only op, Pool is otherwise idle) to keep DVE free.
                for j, i in enumerate(idxs):
                    nc.gpsimd.tensor_scalar(
                        out=xn_outs[i], in0=xts[i],
                        scalar1=mv[:, i, 0:1], scalar2=rstd[:, j : j + 1],
                        op0=ALU.subtract, op1=ALU.mult,
                    )

        def transpose_to(dstT, xn, tok):
            # 4 PE transposes (bf16, 1 cyc/row) of one [128tok, 512E] tile into
            # one psum bank, then a single strided DVE copy (2x mode) into
            # dstT[:, :, tok*128:(tok+1)*128]; used for the latency-critical
            # LN2 path (phase A uses dma_start_transpose instead)
            pt = psum.tile([P, 512], bf16, tag="tp")
            for ec in range(4):
                nc.tensor.transpose(
                    pt[:, ec * P : (ec + 1) * P], xn[:, ec * P : (ec + 1) * P], id128b
                )
            nc.vector.tensor_copy(
                out=dstT[:, :, tok * P : (tok + 1) * P],
                in_=pt.rearrange("p (c t) -> p c t", c=4),
            )

        # ---------------- Phase A: load x, LN1, DMA-transpose to xnT ----------
        # x DMA dispatch alternates SP / ACT queues (both idle early); LN rstd
        # is batched 4 tiles per Ln+Exp pair; transposes run on the DMA
        # engines (dispatch: first half ACT, second half SP) so DVE only
        # carries bn_stats and the q/k/v psum evictions early on.
        def phase_a_group(g4):
            xts, xns = [], []
            for t in range(4 * g4, 4 * g4 + 4):
                if t < 8:
                    xt = xq_sb[:, t, :]
                else:
                    xt = xtp.tile([P, E], fp32, tag="xt", name=f"xt_{t}")
                eng = nc.sync if t % 2 == 0 else nc.scalar
                eng.dma_start(out=xt, in_=x_view[t])
                xts.append(xt)
                xns.append(xtp.tile([P, E], bf16, tag="xn", name=f"xn_{t}"))
            ln_group(xts, xns)
            for i, xn in enumerate(xns):
                t = 4 * g4 + i
                teng = nc.scalar if g4 < 2 else nc.sync
                teng.dma_start_transpose(out=xnT_sb[:, t, :, :], in_=xn)

        phase_a_group(0)
        phase_a_group(1)

        # ---------------- Phase B: QKV matmuls (bf16) ----------------
        # PE issues its stream in order, so any multi-microsecond run of
        # bf16 matmuls starves the ACT exp spine. Only window 0 of K/Q is
        # emitted up front (enough for the first scores); everything else
        # becomes filler closures pumped one-per-kc inside scores blocks.
        kT_x = [kT_a, kT_b]
        qT_x = [qT_a, qT_b]

        def kq_group(w, m, is_k):
            base = 512 if is_k else 0
            dst = (kT_x if is_k else qT_x)[m // 2]
            bias = (bk_sb if is_k else bq_sb)[:, m : m + 1]
            pt = psum.tile([P, 512], fp32, tag="tp")
            for ec in range(4):
                nc.tensor.matmul(
                    pt,
                    lhsT=wqkv_sb[:, ec, base + m * P : base + (m + 1) * P],
                    rhs=xnT_sb[:, 4 * w : 4 * w + 4, ec, :],
                    start=(ec == 0), stop=(ec == 3),
                )
            nc.vector.tensor_scalar_add(
                out=dst[:, m % 2, w * 512 : (w + 1) * 512], in0=pt, scalar1=bias
            )

        def v_group(tcn):
            pt = psum.tile([P, 512], fp32, tag="tp")
            for ec in range(4):
                nc.tensor.matmul(
                    pt,
                    lhsT=xnT_sb[:, tcn, ec, :],
                    rhs=wqkv_sb[:, ec, 1024:1536],
                    start=(ec == 0), stop=(ec == 3),
                )
            nc.vector.tensor_copy(
                out=v_sb[:, tcn, :, :],
                in_=pt.rearrange("p (h d) -> p h d", h=H),
            )

        from collections import deque

        fillers = deque()

        def pump(k=1):
            for _ in range(min(k, len(fillers))):
                fillers.popleft()()

        # ---------------- Phases C/D/E interleaved per query block ----------------
        w1_sb = wpool.tile([P, 4, HID], bf16, tag="w")
        nc.sync.dma_start(out=w1_sb, in_=w1_d[:].rearrange("(c p) n -> p c n", p=P))
        w2_sb = wpool.tile([P, 12, E], fp8, tag="w")
        nc.sync.dma_start(out=w2_sb, in_=w2_d[:].rearrange("(c p) n -> p c n", p=P))

        def scores_block(qb, jh, pump_from=0, pump_k=1):
            # halves[kh][:, kc8, h2, :] = exp(scores/8) fp8 for head 2*jh+h2,
            # keys (kh*4+kc8)*128..+128; pumps filler PE work after each kc
            # so the exp-paced stream never leaves PE with a long stall run
            kt, qt = kT_x[jh // 2], qT_x[jh // 2]
            g0 = 2 * (jh % 2)
            halves = []
            for kh in range(4):
                expSp = expp.tile([P, 4, 2, 512], fp8, tag="es")
                halves.append(expSp)
                for kc8 in range(4):
                    kc = kh * 4 + kc8
                    pt = psum.tile([P, 1024], fp32, tag="sc")
                    for h2 in range(2):
                        g = g0 + h2
                        nc.tensor.matmul(
                            pt[:, h2 * 512 : (h2 + 1) * 512],
                            lhsT=kt[32 * g : 32 * g + 32, :, kc * P : (kc + 1) * P],
                            rhs=qt[32 * g : 32 * g + 32, :, qb * 512 : (qb + 1) * 512],
                            start=True, stop=True, perf_mode=DR,
                            tile_position=(32 * g, 0),
                        )
                    nc.scalar.activation(
                        out=expSp[:, kc8, :, :], in_=pt, func=AF.Exp, scale=HD**-0.5
                    )
                    if kc >= pump_from:
                        pump(pump_k)
            return halves

        def attnv_half(qb, jh, halves, h2):
            h = 2 * jh + h2
            # attnV out in cols 0:512, softmax denominator (32 identical
            # rows from the ones stationary) in cols 512:1024
            pa = psum1.tile([64, 1024], fp32, tag="pa")
            for kc2 in range(8):
                kc = 2 * kc2
                nc.tensor.matmul(
                    pa[:, 0:512],
                    lhsT=v_sb[:, kc : kc + 2, h, :],
                    rhs=halves[kc // 4][:, kc % 4 : kc % 4 + 2, h2, :],
                    start=(kc2 == 0), stop=(kc2 == 7), perf_mode=DR,
                )
            for kc2 in range(8):
                kc = 2 * kc2
                nc.tensor.matmul(
                    pa[0:32, 512:1024],
                    lhsT=ones8,
                    rhs=halves[kc // 4][:, kc % 4 : kc % 4 + 2, h2, :],
                    start=(kc2 == 0), stop=(kc2 == 7), perf_mode=DR,
                    skip_group_check=True,
                )
            ah = work.tile([65, 512], fp32, tag="ah")
            nc.vector.tensor_copy(out=ah[0:64, :], in_=pa[:, 0:512])
            nc.vector.tensor_copy(out=ah[64:65, :], in_=pa[0:1, 512:1024])
            pt2 = psum.tile([P, 4, 65], fp32, tag="tp")
            for qs in range(4):
                nc.tensor.transpose(
                    pt2[:, qs, :], ah[:, qs * P : (qs + 1) * P],
                    id128[0:65, 0:65],
                )
            for qs in range(4):
                tcq = qb * 4 + qs
                rec = work.tile([P, 1], fp32, tag="rec")
                nc.vector.reciprocal(out=rec, in_=pt2[:, qs, 64:65])
                nc.vector.tensor_scalar_mul(
                    out=att_sb[:, tcq, h, :], in0=pt2[:, qs, 0:HD], scalar1=rec
                )

        def attnv_block(qb, jh, halves):
            attnv_half(qb, jh, halves, 0)
            attnv_half(qb, jh, halves, 1)

        def residual_ln2_block(qb, batched=True):
            tcns = list(range(qb * 4, qb * 4 + 4))
            xn2s = []
            for tcn in tcns:
                x2t = xq_sb[:, tcn, :]
                nc.gpsimd.tensor_tensor(
                    out=x2t, in0=x2t,
                    in1=att_sb[:, tcn].rearrange("p h d -> p (h d)"), op=ALU.add,
                )
                nc.gpsimd.tensor_tensor(out=x2t, in0=x2t, in1=bv_bc, op=ALU.add)
                xn2s.append(xtp.tile([P, E], bf16, tag="xn", name=f"xn2_{tcn}"))
            ln_group([xq_sb[:, tcn, :] for tcn in tcns], xn2s, batched=batched)
            for tcn, xn2 in zip(tcns, xn2s):
                transpose_to(xn2T_sb, xn2, tcn)
                # pre-add the fc2 bias into the residual now (LN2 already
                # consumed x2), shortening the final eviction to one add
                nc.gpsimd.tensor_tensor(
                    out=xq_sb[:, tcn, :], in0=xq_sb[:, tcn, :], in1=b2_bc, op=ALU.add
                )

        def fc1_group(qb, mh):
            pt = psum.tile([P, 512], fp32, tag="tp")
            for ec in range(4):
                nc.tensor.matmul(
                    pt,
                    lhsT=w1_sb[:, ec, mh * P : (mh + 1) * P],
                    rhs=xn2T_sb[:, ec, qb * 512 : (qb + 1) * 512],
                    start=(ec == 0), stop=(ec == 3),
                )
            if qb == 0:
                # raw bf16 evict with bias folded in (DVE); one wide gelu
                # later keeps ACT on the exp table during attention
                nc.vector.tensor_scalar_add(
                    out=g1r_sb[:, mh, :], in0=pt, scalar1=b1_sb[:, mh : mh + 1]
                )
            else:
                # tail half: gelu-fused ACT eviction (ACT is free then,
                # DVE is the tail straggler)
                nc.scalar.activation(
                    out=g1T_sb[:, mh, 512:1024], in_=pt,
                    func=AF.Gelu, bias=b1_sb[:, mh : mh + 1], scale=1.0,
                )

        def fc2_block(tcn):
            pt = psum.tile([P, 512], fp32, tag="tp")
            for jp in range(6):
                nc.tensor.matmul(
                    pt,
                    lhsT=g1T_sb[:, 2 * jp : 2 * jp + 2, tcn * P : (tcn + 1) * P],
                    rhs=w2_sb[:, 2 * jp : 2 * jp + 2, :],
                    start=(jp == 0), stop=(jp == 5), perf_mode=DR,
                )
            ot = work.tile([P, E], fp32, tag="ot")
            nc.vector.tensor_tensor(out=ot, in0=pt, in1=xq_sb[:, tcn, :], op=ALU.add)
            # tail tiles fan their DMA dispatch across the idle queues so the
            # last writeback isn't serialized behind seven others on SP
            eng = {4: nc.scalar, 5: nc.gpsimd, 6: nc.scalar, 7: nc.sync}.get(tcn, nc.sync)
            eng.dma_start(out=out_view[tcn], in_=ot)

        # ---- emission order IS the per-engine schedule (in-order issue) ----
        # Up-front PE work: only the quartet-a slots of K/Q window 0 — the
        # minimum for the first scores block. Everything else is drip-fed.
        for m in (0, 1):
            kq_group(0, m, is_k=True)
        for m in (0, 1):
            kq_group(0, m, is_k=False)
        phase_a_group(2)
        phase_a_group(3)
        # Filler order = earliest consumer: quartet-a K windows (blocks
        # (0,0)/(0,1)), then quartet-b K/Q w0 (blocks (0,2)/(0,3)), then Q
        # window 1 (qb=1 blocks). One closure per kc ≈ one exp of ACT time.
        for w in range(1, 4):
            for m in (0, 1):
                fillers.append(lambda w=w, m=m: kq_group(w, m, is_k=True))
        for m in (2, 3):
            fillers.append(lambda m=m: kq_group(0, m, is_k=True))
            fillers.append(lambda m=m: kq_group(0, m, is_k=False))
        for w in range(1, 4):
            for m in (2, 3):
                fillers.append(lambda w=w, m=m: kq_group(w, m, is_k=True))
        for m in range(4):
            fillers.append(lambda m=m: kq_group(1, m, is_k=False))
        h00 = scores_block(0, 0, pump_from=1)
        for tcn in range(16):
            fillers.append(lambda tcn=tcn: v_group(tcn))
        h01 = scores_block(0, 1)
        fillers.append(lambda: attnv_half(0, 0, h00, 0))
        fillers.append(lambda: attnv_half(0, 0, h00, 1))
        fillers.append(lambda: attnv_half(0, 1, h01, 0))
        fillers.append(lambda: attnv_half(0, 1, h01, 1))
        h02 = scores_block(0, 2)
        fillers.append(lambda: attnv_half(0, 2, h02, 0))
        fillers.append(lambda: attnv_half(0, 2, h02, 1))
        h03 = scores_block(0, 3)
        fillers.append(lambda: attnv_half(0, 3, h03, 0))
        fillers.append(lambda: attnv_half(0, 3, h03, 1))
        h10 = scores_block(1, 0, pump_from=1)
        residual_ln2_block(0)  # after sc(1,0): its Ln/Exp never stalls ACT
        fillers.extend(
            lambda mh=mh: fc1_group(0, mh) for mh in range(12)
        )
        fillers.append(lambda: attnv_half(1, 0, h10, 0))
        fillers.append(lambda: attnv_half(1, 0, h10, 1))
        h11 = scores_block(1, 1, pump_from=4, pump_k=2)
        fillers.append(lambda: attnv_half(1, 1, h11, 0))
        fillers.append(lambda: attnv_half(1, 1, h11, 1))
        h12 = scores_block(1, 2, pump_from=1)
        fillers.append(lambda: attnv_half(1, 2, h12, 0))
        fillers.append(lambda: attnv_half(1, 2, h12, 1))
        h13 = scores_block(1, 3, pump_from=1)
        pump(len(fillers))

        # Gate the wide gelu on the LAST exp tile (bypass: data unchanged):
        # ungated, the scheduler hoists it mid-spine (g1r has been ready
        # since fc1(0)) and stretches the exp stream by ~8us + 2 table loads.
        probe = work.tile([P, 1], fp32, tag="probe")
        nc.gpsimd.tensor_copy(out=probe, in_=h13[3][:, 3, 1, 0:1])
        nc.gpsimd.tensor_scalar(
            out=g1r_sb[:, 0, 0:1], in0=g1r_sb[:, 0, 0:1], scalar1=probe,
            scalar2=None, op0=ALU.bypass,
        )
        nc.scalar.activation(
            out=g1T_sb[:, :, 0:512], in_=g1r_sb, func=AF.Gelu, scale=1.0
        )
        attnv_block(1, 3, h13)
        residual_ln2_block(1, batched=False)
        for mh in range(12):
            fc1_group(1, mh)
        for tcn in range(0, 4):
            fc2_block(tcn)
        for tcn in range(4, 8):
            fc2_block(tcn)

    if split_waits:
        _split_matmul_waits(nc, mybir)
    return nc


def _split_matmul_waits(nc, mybir):
    """walrus allows only one sync wait per engine instruction; hoist extra
    waits onto same-engine NoOps placed just before (NX dispatch is in-order,
    so the nops' waits gate the instruction)."""
    k = 0
    for fn in nc.m.functions:
        for blk in fn.blocks:
            new = []
            for inst in blk.instructions:
                si = inst.sync_info
                if si is not None and si.on_wait and len(si.on_wait) > 1:
                    for w in si.on_wait[:-1]:
                        nop = mybir.InstNoOp(name=f"waitnop-{k}", ins=[], outs=[])
                        k += 1
                        nop.engine = inst.engine
                        nop.sync_info = mybir.SyncInfo(on_wait=[w], on_update=[])
                        new.append(nop)
                    inst.sync_info = mybir.SyncInfo(
                        on_wait=[si.on_wait[-1]], on_update=si.on_update
                    )
                new.append(inst)
            blk.instructions[:] = new


def _get_nc():
    if "nc" not in _NC_CACHE:
        _NC_CACHE["nc"] = _build_nc()
    return _NC_CACHE["nc"]


def _qk_slot_perm():
    # perm[j]: source channel (within a heads-major 512-col q or k section)
    # for permuted column j = m*128 + p, m = (quartet x)*2 + slot s; the
    # psum partition p then lands head 4x + p//32, channel 32s + p%32.
    perm = np.empty(512, np.int64)
    for m in range(4):
        x, s = m // 2, m % 2
        for p in range(P):
            perm[m * 128 + p] = (4 * x + p // 32) * 64 + s * 32 + (p % 32)
    return perm


def _prep_inputs(inputs):
    fp8 = ml_dtypes.float8_e4m3
    bf16 = ml_dtypes.bfloat16
    x = np.asarray(inputs["x"], np.float32)
    qkv_w = np.asarray(inputs["qkv_w"], np.float32)
    qkv_b = np.asarray(inputs["qkv_b"], np.float32)
    fc1_w = np.asarray(inputs["fc1_w"], np.float32)
    fc1_b = np.asarray(inputs["fc1_b"], np.float32)
    fc2_w = np.asarray(inputs["fc2_w"], np.float32)
    fc2_b = np.asarray(inputs["fc2_b"], np.float32)

    # reorder qkv channels: per-head interleave [q|k|v]*H -> heads-major
    # [Q|K|V], then slot-permute Q and K columns for the DoubleRow layout
    w3 = qkv_w.reshape(E, H, 3, HD)
    wq, wk, wv = (w3[:, :, i, :].reshape(E, E) for i in range(3))
    b3 = qkv_b.reshape(H, 3, HD)
    bq, bk, bv = (np.ascontiguousarray(b3[:, i, :].reshape(E)) for i in range(3))

    perm = _qk_slot_perm()
    wqkv = np.ascontiguousarray(
        np.concatenate([wq[:, perm], wk[:, perm], wv], axis=1)
    ).astype(bf16)
    bq_t = np.ascontiguousarray(bq[perm].reshape(4, P).T)  # [P, 4]
    bk_t = np.ascontiguousarray(bk[perm].reshape(4, P).T)

    w1 = np.ascontiguousarray(fc1_w).astype(bf16)
    w2 = np.ascontiguousarray(fc2_w).astype(fp8)

    in_maps = []
    for c in range(NCORES):
        b, half = c // 2, c % 2
        xr = np.ascontiguousarray(np.roll(x[b], -half * NQ, axis=0))
        in_maps.append(
            {
                "x": xr,
                "wqkv": wqkv,
                "bq": bq_t,
                "bk": bk_t,
                "bv": bv,
                "w1": w1,
                "b1": fc1_b,
                "w2": w2,
                "b2": fc2_b,
            }
        )
    return in_maps


def kernel(**inputs) -> np.ndarray:
    from concourse.bass_utils import run_bass_kernel_spmd

    nc = _get_nc()
    in_maps = _prep_inputs(inputs)
    res = run_bass_kernel_spmd(nc, in_maps, core_ids=list(range(NCORES)))
    y = np.empty((B, N, E), np.float32)
    for c in range(NCORES):
        b, half = c // 2, c % 2
        y[b, half * NQ : (half + 1) * NQ] = np.asarray(res.results[c]["out"])
    return y


if __name__ == "__main__":
    nc = _build_nc()
    print("build OK")


# revision 58
# speedup vs baseline: 1.2759x; 1.0043x over previous
"""Trainium2 Bass kernel for nn_EncoderLayer (B=4, N=2048, E=512, H=8, HIDDEN=1536).

Sharding: 8 cores; core c handles batch b=c//2, query-half c%2 (1024 query
rows). Each core computes K/V over the full 2048-row sequence of its batch
(keys are permutation-invariant under softmax, so the host rotates x[b] to put
the query rows first), and the FFN over its 1024 rows only.

Precision (driven by an error-attribution study against the fp32 reference):
attention (q/k/v, exp(scores), attnV) and fc2 run in fp8e4m3 DoubleRow perf
mode (2 reduction rows/partition, 0.5 cyc/row) — their quantization noise
washes out against the residual path. QKV projection and fc1 stay bf16 (fc1
error feeds gelu+fc2 directly and dominated the budget). Residual/LN fp32.

Layouts: per-head qk contraction ch -> (partition 32*g + ch%32, slot ch//32),
head quartets a=(0..3)/b=(4..7) stacked along partitions (host permutes wqkv
q/k columns so PSUM evictions land partition-aligned). fc2 contraction
hc -> (pair jp = hc//256, slot (hc//128)%2, p = hc%128) falls out of the
[P, 12, NQ] g1 layout for free.

Engine split: PE matmuls + attention-out transposes; ACT exp spine + gelu
(deferred/fused to bracket one table switch) + LN rstd via Ln/Exp (exp-table
resident); DVE LN stats, psum evictions, softmax scale; Pool (idle otherwise)
LN applies and residual adds; DMA engines do the LN1 transposes
(dma_start_transpose), keeping DVE off the phase-A critical path.
"""

import sys

sys.path.insert(0, "/opt/trn_rl_repo")

import numpy as np
import ml_dtypes

B, N, E = 4, 2048, 512
H, HD = 8, 64
HID = 3 * E
NQ = 1024  # query rows per core
P = 128
EPS = 1e-5
NCORES = 8

_NC_CACHE = {}


def _build_nc(split_waits=True):
    from contextlib import ExitStack

    import concourse.bass as bass
    import concourse.mybir as mybir
    import concourse.tile as tile
    from concourse.masks import make_identity

    fp32 = mybir.dt.float32
    bf16 = mybir.dt.bfloat16
    fp8 = mybir.dt.float8e4
    AF = mybir.ActivationFunctionType
    ALU = mybir.AluOpType
    DR = mybir.MatmulPerfMode.DoubleRow

    nc = bass.Bass()

    x_d = nc.declare_dram_parameter("x", [N, E], fp32, isOutput=False)
    wqkv_d = nc.declare_dram_parameter("wqkv", [E, 3 * E], bf16, isOutput=False)
    bq_d = nc.declare_dram_parameter("bq", [P, 4], fp32, isOutput=False)
    bk_d = nc.declare_dram_parameter("bk", [P, 4], fp32, isOutput=False)
    bv_d = nc.declare_dram_parameter("bv", [E], fp32, isOutput=False)
    w1_d = nc.declare_dram_parameter("w1", [E, HID], bf16, isOutput=False)
    b1_d = nc.declare_dram_parameter("b1", [HID], fp32, isOutput=False)
    w2_d = nc.declare_dram_parameter("w2", [HID, E], fp8, isOutput=False)
    b2_d = nc.declare_dram_parameter("b2", [E], fp32, isOutput=False)
    out_d = nc.declare_dram_parameter("out", [NQ, E], fp32, isOutput=True)

    x_view = x_d[:].rearrange("(t p) e -> t p e", p=P)  # [16, 128, 512]
    out_view = out_d[:].rearrange("(t p) e -> t p e", p=P)  # [8, 128, 512]

    def bcast(ap, parts=P):
        return bass.AP(tensor=ap.tensor, offset=ap.offset, ap=[[0, parts]] + list(ap.ap))

    with tile.TileContext(nc) as tc, ExitStack() as ctx:
        const = ctx.enter_context(tc.tile_pool(name="const", bufs=1))
        big = ctx.enter_context(tc.tile_pool(name="big", bufs=1))
        wpool = ctx.enter_context(tc.tile_pool(name="wpool", bufs=2))
        work = ctx.enter_context(tc.tile_pool(name="work", bufs=3))
        xtp = ctx.enter_context(tc.tile_pool(name="xtp", bufs=4))
        expp = ctx.enter_context(tc.tile_pool(name="expp", bufs=12))
        psum = ctx.enter_context(tc.tile_pool(name="psum", bufs=2, space="PSUM"))
        psum1 = ctx.enter_context(tc.tile_pool(name="psum1", bufs=1, space="PSUM"))

        id128 = const.tile([P, P], fp32)
        make_identity(nc, id128)
        id128b = const.tile([P, P], bf16)
        nc.vector.tensor_copy(out=id128b, in_=id128)
        eps_sb = const.tile([P, 1], fp32)
        nc.vector.memset(eps_sb, EPS)
        # DoubleRow ones for the softmax denominator (DR psum outputs must
        # start at partition 0, so the denominator rides spare pa columns)
        ones8 = const.tile([P, 2, 32], fp8)
        nc.vector.memset(ones8, 1.0)

        bq_sb = const.tile([P, 4], fp32)
        nc.sync.dma_start(out=bq_sb, in_=bq_d[:])
        bk_sb = const.tile([P, 4], fp32)
        nc.sync.dma_start(out=bk_sb, in_=bk_d[:])
        bv_bc = const.tile([P, E], fp32)
        nc.sync.dma_start(out=bv_bc, in_=bcast(bv_d[:]))
        b1_sb = const.tile([P, 12], fp32)
        nc.sync.dma_start(out=b1_sb, in_=b1_d[:].rearrange("(c p) -> p c", p=P))
        b2_bc = const.tile([P, E], fp32)
        nc.sync.dma_start(out=b2_bc, in_=bcast(b2_d[:]))

        # wpool tag "w": two slots rotating through wqkv -> xnT -> w1 -> w2
        # (xnT is dead after phase B+V)
        wqkv_sb = wpool.tile([P, 4, 3 * E], bf16, tag="w")
        nc.sync.dma_start(out=wqkv_sb, in_=wqkv_d[:].rearrange("(c p) n -> p c n", p=P))
        # LN1(x) transposed, token-block-major so each dma_start_transpose
        # writes a contiguous [P, 4, 128] destination
        xnT_sb = wpool.tile([P, 16, 4, P], bf16, tag="w")

        xq_sb = big.tile([P, 8, E], fp32)       # raw x query rows; becomes x2 in place
        qT_a = big.tile([P, 2, NQ], fp8)        # heads 0-3, partition 32g+c, slot s
        qT_b = big.tile([P, 2, NQ], fp8)        # heads 4-7
        kT_a = big.tile([P, 2, N], fp8)
        kT_b = big.tile([P, 2, N], fp8)
        v_sb = big.tile([P, 16, H, HD], fp8)    # token-major V
        att_sb = big.tile([P, 8, H, HD], bf16)
        xn2T_sb = big.tile([P, 4, NQ], bf16)
        g1r_sb = big.tile([P, 12, 512], bf16)   # fc1(qb=0) raw (bias added)
        g1T_sb = big.tile([P, 12, NQ], fp8)

        def ln_group(xts, xn_outs, batched=True, apply_eng=None):
            # LN over up to 4 token tiles; batched=True shares one Ln+Exp
            # pair across the group (fewer ACT instrs, +latency), while
            # batched=False pipelines per tile (for the latency-critical
            # tail). rstd via exp(-0.5*ln(var+eps)) keeps ACT on the
            # natural_log_exp table set (shared with softmax exp) — no
            # table switching against the attention exp stream.
            n = len(xts)
            groups = [range(n)] if batched else [[i] for i in range(n)]
            mv = work.tile([P, n, 2], fp32, tag="mv")
            for idxs in groups:
                for i in idxs:
                    stats = work.tile([P, 6], fp32, tag="st")
                    nc.vector.bn_stats(out=stats, in_=xts[i])
                    nc.vector.bn_aggr(out=mv[:, i, :], in_=stats)
                i0, ng = idxs[0], len(idxs)
                lnv = work.tile([P, ng], fp32, tag="lnv")
                nc.scalar.activation(
                    out=lnv, in_=mv[:, i0 : i0 + ng, 1], func=AF.Ln, bias=eps_sb, scale=1.0
                )
                rstd = work.tile([P, ng], fp32, tag="rstd")
                nc.scalar.activation(out=rstd, in_=lnv, func=AF.Exp, scale=-0.5)
                # NOTE: ACT Identity with a per-partition scale AP crashes the
                # device (NRT_EXEC_UNIT_UNRECOVERABLE). The apply runs on Pool
                # (SBUF-only op, Pool is otherwise idle) to keep DVE free.
                for j, i in enumerate(idxs):
                    (apply_eng or nc.gpsimd).tensor_scalar(
                        out=xn_outs[i], in0=xts[i],
                        scalar1=mv[:, i, 0:1], scalar2=rstd[:, j : j + 1],
                        op0=ALU.subtract, op1=ALU.mult,
                    )

        def transpose_to(dstT, xn, tok):
            # 4 PE transposes (bf16, 1 cyc/row) of one [128tok, 512E] tile into
            # one psum bank, then a single strided DVE copy (2x mode) into
            # dstT[:, :, tok*128:(tok+1)*128]; used for the latency-critical
            # LN2 path (phase A uses dma_start_transpose instead)
            pt = psum.tile([P, 512], bf16, tag="tp")
            for ec in range(4):
                nc.tensor.transpose(
                    pt[:, ec * P : (ec + 1) * P], xn[:, ec * P : (ec + 1) * P], id128b
                )
            nc.vector.tensor_copy(
                out=dstT[:, :, tok * P : (tok + 1) * P],
                in_=pt.rearrange("p (c t) -> p c t", c=4),
            )

        # ---------------- Phase A: load x, LN1, DMA-transpose to xnT ----------
        # x DMA dispatch alternates SP / ACT queues (both idle early); LN rstd
        # is batched 4 tiles per Ln+Exp pair; transposes run on the DMA
        # engines (dispatch: first half ACT, second half SP) so DVE only
        # carries bn_stats and the q/k/v psum evictions early on.
        def phase_a_group(g4):
            xts, xns = [], []
            for t in range(4 * g4, 4 * g4 + 4):
                if t < 8:
                    xt = xq_sb[:, t, :]
                else:
                    xt = xtp.tile([P, E], fp32, tag="xt", name=f"xt_{t}")
                eng = nc.sync if t % 2 == 0 else nc.scalar
                eng.dma_start(out=xt, in_=x_view[t])
                xts.append(xt)
                xns.append(xtp.tile([P, E], bf16, tag="xn", name=f"xn_{t}"))
            ln_group(xts, xns)
            for i, xn in enumerate(xns):
                t = 4 * g4 + i
                if g4 == 0:
                    # group 0 is on the first-exp critical path: the PE+DVE
                    # transpose route is ~1.5us lower latency than the
                    # dma-transpose round trip
                    pt = psum.tile([P, 512], bf16, tag="tp")
                    for ec in range(4):
                        nc.tensor.transpose(
                            pt[:, ec * P : (ec + 1) * P],
                            xn[:, ec * P : (ec + 1) * P], id128b,
                        )
                    nc.vector.tensor_copy(
                        out=xnT_sb[:, t, :, :],
                        in_=pt.rearrange("p (c t) -> p c t", c=4),
                    )
                else:
                    teng = nc.scalar if g4 == 1 else nc.sync
                    teng.dma_start_transpose(out=xnT_sb[:, t, :, :], in_=xn)

        phase_a_group(0)
        phase_a_group(1)

        # ---------------- Phase B: QKV matmuls (bf16) ----------------
        # PE issues its stream in order, so any multi-microsecond run of
        # bf16 matmuls starves the ACT exp spine. Only window 0 of K/Q is
        # emitted up front (enough for the first scores); everything else
        # becomes filler closures pumped one-per-kc inside scores blocks.
        kT_x = [kT_a, kT_b]
        qT_x = [qT_a, qT_b]

        def kq_group(w, m, is_k, evict_on_act=False):
            base = 512 if is_k else 0
            dst = (kT_x if is_k else qT_x)[m // 2]
            bias = (bk_sb if is_k else bq_sb)[:, m : m + 1]
            pt = psum.tile([P, 512], fp32, tag="tp")
            for ec in range(4):
                nc.tensor.matmul(
                    pt,
                    lhsT=wqkv_sb[:, ec, base + m * P : base + (m + 1) * P],
                    rhs=xnT_sb[:, 4 * w : 4 * w + 4, ec, :],
                    start=(ec == 0), stop=(ec == 3),
                )
            out = dst[:, m % 2, w * 512 : (w + 1) * 512]
            if evict_on_act:
                # prologue evictions ride the idle ACT engine (Identity is
                # resident in every activation table — no load)
                nc.scalar.activation(out=out, in_=pt, func=AF.Identity, bias=bias, scale=1.0)
            else:
                nc.vector.tensor_scalar_add(out=out, in0=pt, scalar1=bias)

        def v_group(tcn):
            pt = psum.tile([P, 512], fp32, tag="tp")
            for ec in range(4):
                nc.tensor.matmul(
                    pt,
                    lhsT=xnT_sb[:, tcn, ec, :],
                    rhs=wqkv_sb[:, ec, 1024:1536],
                    start=(ec == 0), stop=(ec == 3),
                )
            # the v bias rides the eviction (softmax rows sum to 1, so
            # attnV(v + bv) = attnV(v) + bv — no separate post-add needed)
            nc.vector.tensor_tensor(
                out=v_sb[:, tcn, :, :],
                in0=pt.rearrange("p (h d) -> p h d", h=H),
                in1=bv_bc.rearrange("p (h d) -> p h d", h=H),
                op=ALU.add,
            )

        from collections import deque

        fillers = deque()

        def pump(k=1):
            for _ in range(min(k, len(fillers))):
                fillers.popleft()()

        # ---------------- Phases C/D/E interleaved per query block ----------------
        w1_sb = wpool.tile([P, 4, HID], bf16, tag="w")
        nc.sync.dma_start(out=w1_sb, in_=w1_d[:].rearrange("(c p) n -> p c n", p=P))
        w2_sb = wpool.tile([P, 12, E], fp8, tag="w")
        nc.sync.dma_start(out=w2_sb, in_=w2_d[:].rearrange("(c p) n -> p c n", p=P))

        def scores_block(qb, jh, pump_from=0, pump_k=1):
            # halves[kh][:, kc8, h2, :] = exp(scores/8) fp8 for head 2*jh+h2,
            # keys (kh*4+kc8)*128..+128; pumps filler PE work after each kc
            # so the exp-paced stream never leaves PE with a long stall run
            kt, qt = kT_x[jh // 2], qT_x[jh // 2]
            g0 = 2 * (jh % 2)
            halves = []
            for kh in range(4):
                expSp = expp.tile([P, 4, 2, 512], fp8, tag="es")
                halves.append(expSp)
                for kc8 in range(4):
                    kc = kh * 4 + kc8
                    pt = psum.tile([P, 1024], fp32, tag="sc")
                    for h2 in range(2):
                        g = g0 + h2
                        nc.tensor.matmul(
                            pt[:, h2 * 512 : (h2 + 1) * 512],
                            lhsT=kt[32 * g : 32 * g + 32, :, kc * P : (kc + 1) * P],
                            rhs=qt[32 * g : 32 * g + 32, :, qb * 512 : (qb + 1) * 512],
                            start=True, stop=True, perf_mode=DR,
                            tile_position=(32 * g, 0),
                        )
                    nc.scalar.activation(
                        out=expSp[:, kc8, :, :], in_=pt, func=AF.Exp, scale=HD**-0.5
                    )
                    if kc >= pump_from:
                        pump(pump_k)
            return halves

        def attnv_parts(qb, jh, halves, h2):
            # split into two pumpable closures (~0.9us PE each) so a single
            # attnV burst never delays the next scores matmul by >1 exp
            h = 2 * jh + h2
            state = {}

            def emit_mm(pa, lo, hi):
                for kc2 in range(lo, hi):
                    kc = 2 * kc2
                    rhs = halves[kc // 4][:, kc % 4 : kc % 4 + 2, h2, :]
                    # attnV out in cols 0:512, softmax denominator (32
                    # identical rows from the ones stationary) in 512:1024
                    nc.tensor.matmul(
                        pa[:, 0:512],
                        lhsT=v_sb[:, kc : kc + 2, h, :], rhs=rhs,
                        start=(kc2 == 0), stop=(kc2 == 7), perf_mode=DR,
                    )
                    nc.tensor.matmul(
                        pa[0:32, 512:1024],
                        lhsT=ones8, rhs=rhs,
                        start=(kc2 == 0), stop=(kc2 == 7), perf_mode=DR,
                        skip_group_check=True,
                    )

            def mm1():
                pa = psum1.tile([64, 1024], fp32, tag="pa", name=f"pa_{qb}{jh}{h2}")
                state["pa"] = pa
                emit_mm(pa, 0, 4)

            def mm2():
                pa = state["pa"]
                emit_mm(pa, 4, 8)
                ah = work.tile([65, 512], fp32, tag="ah", name=f"ah_{qb}{jh}{h2}")
                nc.vector.tensor_copy(out=ah[0:64, :], in_=pa[:, 0:512])
                nc.vector.tensor_copy(out=ah[64:65, :], in_=pa[0:1, 512:1024])
                pt2 = psum.tile([P, 4, 65], fp32, tag="tp", name=f"pt2_{qb}{jh}{h2}")
                for qs in range(4):
                    nc.tensor.transpose(
                        pt2[:, qs, :], ah[:, qs * P : (qs + 1) * P],
                        id128[0:65, 0:65],
                    )
                for qs in range(4):
                    tcq = qb * 4 + qs
                    rec = work.tile([P, 1], fp32, tag="rec", name=f"rec_{qb}{jh}{h2}{qs}")
                    nc.vector.reciprocal(out=rec, in_=pt2[:, qs, 64:65])
                    nc.vector.tensor_scalar_mul(
                        out=att_sb[:, tcq, h, :], in0=pt2[:, qs, 0:HD], scalar1=rec
                    )

            return mm1, mm2

        def attnv_enqueue(qb, jh, halves):
            for h2 in range(2):
                mm1, mm2 = attnv_parts(qb, jh, halves, h2)
                fillers.append(mm1)
                fillers.append(mm2)

        def attnv_block(qb, jh, halves):
            for h2 in range(2):
                mm1, mm2 = attnv_parts(qb, jh, halves, h2)
                mm1()
                mm2()

        def residual_ln2_block(qb, batched=True):
            # the tail block (qb=1) keeps everything on DVE: cross-engine
            # sem hops cost more than DVE serialization on the critical tail
            tcns = list(range(qb * 4, qb * 4 + 4))
            add_eng = nc.gpsimd if batched else nc.vector
            xn2s = []
            for tcn in tcns:
                x2t = xq_sb[:, tcn, :]
                add_eng.tensor_tensor(
                    out=x2t, in0=x2t,
                    in1=att_sb[:, tcn].rearrange("p h d -> p (h d)"), op=ALU.add,
                )
                xn2s.append(xtp.tile([P, E], bf16, tag="xn", name=f"xn2_{tcn}"))
            ln_group(
                [xq_sb[:, tcn, :] for tcn in tcns], xn2s,
                batched=batched, apply_eng=add_eng,
            )
            for tcn, xn2 in zip(tcns, xn2s):
                transpose_to(xn2T_sb, xn2, tcn)
                # pre-add the fc2 bias into the residual now (LN2 already
                # consumed x2), shortening the final eviction to one add
                nc.gpsimd.tensor_tensor(
                    out=xq_sb[:, tcn, :], in0=xq_sb[:, tcn, :], in1=b2_bc, op=ALU.add
                )

        def fc1_group(qb, mh):
            pt = psum.tile([P, 512], fp32, tag="tp")
            if qb == 0:
                for ec in range(4):
                    nc.tensor.matmul(
                        pt,
                        lhsT=w1_sb[:, ec, mh * P : (mh + 1) * P],
                        rhs=xn2T_sb[:, ec, 0:512],
                        start=(ec == 0), stop=(ec == 3),
                    )
            else:
                # tail half: token-tile column chunks so each matmul only
                # waits its own xn2T tile (the last LN2 tile gates 1/4 of
                # the psum instead of all of it)
                for tq in range(4):
                    cols = slice((4 + tq) * P, (5 + tq) * P)
                    for ec in range(4):
                        nc.tensor.matmul(
                            pt[:, tq * P : (tq + 1) * P],
                            lhsT=w1_sb[:, ec, mh * P : (mh + 1) * P],
                            rhs=xn2T_sb[:, ec, cols],
                            start=(ec == 0), stop=(ec == 3),
                        )
            if qb == 0:
                # raw bf16 evict with bias folded in (DVE); one wide gelu
                # later keeps ACT on the exp table during attention
                nc.vector.tensor_scalar_add(
                    out=g1r_sb[:, mh, :], in0=pt, scalar1=b1_sb[:, mh : mh + 1]
                )
            else:
                # tail half: gelu-fused ACT eviction (ACT is free then,
                # DVE is the tail straggler)
                nc.scalar.activation(
                    out=g1T_sb[:, mh, 512:1024], in_=pt,
                    func=AF.Gelu, bias=b1_sb[:, mh : mh + 1], scale=1.0,
                )

        def fc2_block(tcn):
            pt = psum.tile([P, 512], fp32, tag="tp")
            for jp in range(6):
                nc.tensor.matmul(
                    pt,
                    lhsT=g1T_sb[:, 2 * jp : 2 * jp + 2, tcn * P : (tcn + 1) * P],
                    rhs=w2_sb[:, 2 * jp : 2 * jp + 2, :],
                    start=(jp == 0), stop=(jp == 5), perf_mode=DR,
                )
            ot = work.tile([P, E], fp32, tag="ot")
            nc.vector.tensor_tensor(out=ot, in0=pt, in1=xq_sb[:, tcn, :], op=ALU.add)
            # tail tiles fan their DMA dispatch across the idle queues so the
            # last writeback isn't serialized behind seven others on SP
            eng = {4: nc.scalar, 5: nc.gpsimd, 6: nc.scalar, 7: nc.sync}.get(tcn, nc.sync)
            eng.dma_start(out=out_view[tcn], in_=ot)

        # ---- emission order IS the per-engine schedule (in-order issue) ----
        # Up-front PE work: only the quartet-a slots of K/Q window 0 — the
        # minimum for the first scores block. Everything else is drip-fed.
        for m in (0, 1):
            kq_group(0, m, is_k=True, evict_on_act=True)
        for m in (0, 1):
            kq_group(0, m, is_k=False, evict_on_act=True)
        phase_a_group(2)
        phase_a_group(3)
        # Filler order = earliest consumer: quartet-a K windows (blocks
        # (0,0)/(0,1)), then quartet-b K/Q w0 (blocks (0,2)/(0,3)), then Q
        # window 1 (qb=1 blocks). One closure per kc ≈ one exp of ACT time.
        for w in range(1, 4):
            for m in (0, 1):
                fillers.append(
                    lambda w=w, m=m: kq_group(w, m, is_k=True, evict_on_act=(w == 1))
                )
        for m in (2, 3):
            fillers.append(lambda m=m: kq_group(0, m, is_k=True))
            fillers.append(lambda m=m: kq_group(0, m, is_k=False))
        for w in range(1, 4):
            for m in (2, 3):
                fillers.append(lambda w=w, m=m: kq_group(w, m, is_k=True))
        for m in range(4):
            fillers.append(lambda m=m: kq_group(1, m, is_k=False))
        h00 = scores_block(0, 0, pump_from=1)
        for tcn in range(16):
            fillers.append(lambda tcn=tcn: v_group(tcn))
        h01 = scores_block(0, 1)
        attnv_enqueue(0, 0, h00)
        attnv_enqueue(0, 1, h01)
        h02 = scores_block(0, 2)
        attnv_enqueue(0, 2, h02)
        h03 = scores_block(0, 3)
        attnv_enqueue(0, 3, h03)
        h10 = scores_block(1, 0, pump_from=1)
        residual_ln2_block(0)  # after sc(1,0): its Ln/Exp never stalls ACT
        fillers.extend(
            lambda mh=mh: fc1_group(0, mh) for mh in range(12)
        )
        attnv_enqueue(1, 0, h10)
        h11 = scores_block(1, 1, pump_from=4, pump_k=2)
        attnv_enqueue(1, 1, h11)
        h12 = scores_block(1, 2, pump_from=1)
        attnv_enqueue(1, 2, h12)
        h13 = scores_block(1, 3, pump_from=1)
        pump(len(fillers))

        # Gate the wide gelu on the LAST exp tile (bypass: data unchanged):
        # ungated, the scheduler hoists it mid-spine (g1r has been ready
        # since fc1(0)) and stretches the exp stream by ~8us + 2 table loads.
        probe = work.tile([P, 1], fp32, tag="probe")
        nc.gpsimd.tensor_copy(out=probe, in_=h13[3][:, 3, 1, 0:1])
        nc.gpsimd.tensor_scalar(
            out=g1r_sb[:, 0, 0:1], in0=g1r_sb[:, 0, 0:1], scalar1=probe,
            scalar2=None, op0=ALU.bypass,
        )
        nc.scalar.activation(
            out=g1T_sb[:, :, 0:512], in_=g1r_sb, func=AF.Gelu, scale=1.0
        )
        attnv_block(1, 3, h13)
        residual_ln2_block(1, batched=False)
        for mh in range(12):
            fc1_group(1, mh)
        for tcn in range(0, 4):
            fc2_block(tcn)
        for tcn in range(4, 8):
            fc2_block(tcn)

    if split_waits:
        _split_matmul_waits(nc, mybir)
    return nc


def _split_matmul_waits(nc, mybir):
    """walrus allows only one sync wait per engine instruction; hoist extra
    waits onto same-engine NoOps placed just before (NX dispatch is in-order,
    so the nops' waits gate the instruction)."""
    k = 0
    for fn in nc.m.functions:
        for blk in fn.blocks:
            new = []
            for inst in blk.instructions:
                si = inst.sync_info
                if si is not None and si.on_wait and len(si.on_wait) > 1:
                    for w in si.on_wait[:-1]:
                        nop = mybir.InstNoOp(name=f"waitnop-{k}", ins=[], outs=[])
                        k += 1
                        nop.engine = inst.engine
                        nop.sync_info = mybir.SyncInfo(on_wait=[w], on_update=[])
                        new.append(nop)
                    inst.sync_info = mybir.SyncInfo(
                        on_wait=[si.on_wait[-1]], on_update=si.on_update
                    )
                new.append(inst)
            blk.instructions[:] = new


def _get_nc():
    if "nc" not in _NC_CACHE:
        _NC_CACHE["nc"] = _build_nc()
    return _NC_CACHE["nc"]


def _qk_slot_perm():
    # perm[j]: source channel (within a heads-major 512-col q or k section)
    # for permuted column j = m*128 + p, m = (quartet x)*2 + slot s; the
    # psum partition p then lands head 4x + p//32, channel 32s + p%32.
    perm = np.empty(512, np.int64)
    for m in range(4):
        x, s = m // 2, m % 2
        for p in range(P):
            perm[m * 128 + p] = (4 * x + p // 32) * 64 + s * 32 + (p % 32)
    return perm


def _prep_inputs(inputs):
    fp8 = ml_dtypes.float8_e4m3
    bf16 = ml_dtypes.bfloat16
    x = np.asarray(inputs["x"], np.float32)
    qkv_w = np.asarray(inputs["qkv_w"], np.float32)
    qkv_b = np.asarray(inputs["qkv_b"], np.float32)
    fc1_w = np.asarray(inputs["fc1_w"], np.float32)
    fc1_b = np.asarray(inputs["fc1_b"], np.float32)
    fc2_w = np.asarray(inputs["fc2_w"], np.float32)
    fc2_b = np.asarray(inputs["fc2_b"], np.float32)

    # reorder qkv channels: per-head interleave [q|k|v]*H -> heads-major
    # [Q|K|V], then slot-permute Q and K columns for the DoubleRow layout
    w3 = qkv_w.reshape(E, H, 3, HD)
    wq, wk, wv = (w3[:, :, i, :].reshape(E, E) for i in range(3))
    b3 = qkv_b.reshape(H, 3, HD)
    bq, bk, bv = (np.ascontiguousarray(b3[:, i, :].reshape(E)) for i in range(3))

    perm = _qk_slot_perm()
    wqkv = np.ascontiguousarray(
        np.concatenate([wq[:, perm], wk[:, perm], wv], axis=1)
    ).astype(bf16)
    bq_t = np.ascontiguousarray(bq[perm].reshape(4, P).T)  # [P, 4]
    bk_t = np.ascontiguousarray(bk[perm].reshape(4, P).T)

    w1 = np.ascontiguousarray(fc1_w).astype(bf16)
    w2 = np.ascontiguousarray(fc2_w).astype(fp8)

    in_maps = []
    for c in range(NCORES):
        b, half = c // 2, c % 2
        xr = np.ascontiguousarray(np.roll(x[b], -half * NQ, axis=0))
        in_maps.append(
            {
                "x": xr,
                "wqkv": wqkv,
                "bq": bq_t,
                "bk": bk_t,
                "bv": bv,
                "w1": w1,
                "b1": fc1_b,
                "w2": w2,
                "b2": fc2_b,
            }
        )
    return in_maps


def kernel(**inputs) -> np.ndarray:
    from concourse.bass_utils import run_bass_kernel_spmd

    nc = _get_nc()
    in_maps = _prep_inputs(inputs)
    res = run_bass_kernel_spmd(nc, in_maps, core_ids=list(range(NCORES)))
    y = np.empty((B, N, E), np.float32)
    for c in range(NCORES):
        b, half = c // 2, c % 2
        y[b, half * NQ : (half + 1) * NQ] = np.asarray(res.results[c]["out"])
    return y


if __name__ == "__main__":
    nc = _build_nc()
    print("build OK")


# revision 63
# speedup vs baseline: 1.3245x; 1.0381x over previous
"""Trainium2 Bass kernel for nn_EncoderLayer (B=4, N=2048, E=512, H=8, HIDDEN=1536).

Sharding: 8 cores; core c handles batch b=c//2, query-half c%2 (1024 query
rows). Each core computes K/V over the full 2048-row sequence of its batch
(keys are permutation-invariant under softmax, so the host rotates x[b] to put
the query rows first), and the FFN over its 1024 rows only.

Precision (driven by an error-attribution study against the fp32 reference):
attention (q/k/v, exp(scores), attnV) and fc2 run in fp8e4m3 DoubleRow perf
mode (2 reduction rows/partition, 0.5 cyc/row) — their quantization noise
washes out against the residual path. QKV projection and fc1 stay bf16 (fc1
error feeds gelu+fc2 directly and dominated the budget). Residual/LN fp32.

Layouts: per-head qk contraction ch -> (partition 32*g + ch%32, slot ch//32),
head quartets a=(0..3)/b=(4..7) stacked along partitions (host permutes wqkv
q/k columns so PSUM evictions land partition-aligned). fc2 contraction
hc -> (pair jp = hc//256, slot (hc//128)%2, p = hc%128) falls out of the
[P, 12, NQ] g1 layout for free.

Engine split: PE matmuls + attention-out transposes; ACT exp spine + gelu
(deferred/fused to bracket one table switch) + LN rstd via Ln/Exp (exp-table
resident); DVE LN stats, psum evictions, softmax scale; Pool (idle otherwise)
LN applies and residual adds; DMA engines do the LN1 transposes
(dma_start_transpose), keeping DVE off the phase-A critical path.
"""

import sys

sys.path.insert(0, "/opt/trn_rl_repo")

import numpy as np
import ml_dtypes

B, N, E = 4, 2048, 512
H, HD = 8, 64
HID = 3 * E
NQ = 1024  # query rows per core
P = 128
EPS = 1e-5
NCORES = 8

_NC_CACHE = {}


def _build_nc(split_waits=True):
    from contextlib import ExitStack

    import concourse.bass as bass
    import concourse.mybir as mybir
    import concourse.tile as tile
    from concourse.masks import make_identity

    fp32 = mybir.dt.float32
    bf16 = mybir.dt.bfloat16
    fp8 = mybir.dt.float8e4
    AF = mybir.ActivationFunctionType
    ALU = mybir.AluOpType
    DR = mybir.MatmulPerfMode.DoubleRow

    nc = bass.Bass()

    x_d = nc.declare_dram_parameter("x", [N, E], fp32, isOutput=False)
    wqkv_d = nc.declare_dram_parameter("wqkv", [E, 3 * E], bf16, isOutput=False)
    bq_d = nc.declare_dram_parameter("bq", [P, 4], fp32, isOutput=False)
    bk_d = nc.declare_dram_parameter("bk", [P, 4], fp32, isOutput=False)
    bv_d = nc.declare_dram_parameter("bv", [E], fp32, isOutput=False)
    w1_d = nc.declare_dram_parameter("w1", [E, HID], bf16, isOutput=False)
    b1_d = nc.declare_dram_parameter("b1", [HID], fp32, isOutput=False)
    w2_d = nc.declare_dram_parameter("w2", [HID, E], fp8, isOutput=False)
    b2_d = nc.declare_dram_parameter("b2", [E], fp32, isOutput=False)
    out_d = nc.declare_dram_parameter("out", [NQ, E], fp32, isOutput=True)

    x_view = x_d[:].rearrange("(t p) e -> t p e", p=P)  # [16, 128, 512]
    out_view = out_d[:].rearrange("(t p) e -> t p e", p=P)  # [8, 128, 512]

    def bcast(ap, parts=P):
        return bass.AP(tensor=ap.tensor, offset=ap.offset, ap=[[0, parts]] + list(ap.ap))

    with tile.TileContext(nc) as tc, ExitStack() as ctx:
        const = ctx.enter_context(tc.tile_pool(name="const", bufs=1))
        big = ctx.enter_context(tc.tile_pool(name="big", bufs=1))
        wpool = ctx.enter_context(tc.tile_pool(name="wpool", bufs=2))
        work = ctx.enter_context(tc.tile_pool(name="work", bufs=3))
        xtp = ctx.enter_context(tc.tile_pool(name="xtp", bufs=4))
        expp = ctx.enter_context(tc.tile_pool(name="expp", bufs=12))
        psum = ctx.enter_context(tc.tile_pool(name="psum", bufs=2, space="PSUM"))
        psum1 = ctx.enter_context(tc.tile_pool(name="psum1", bufs=1, space="PSUM"))

        id128 = const.tile([P, P], fp32)
        make_identity(nc, id128)
        id128b = const.tile([P, P], bf16)
        nc.vector.tensor_copy(out=id128b, in_=id128)
        eps_sb = const.tile([P, 1], fp32)
        nc.vector.memset(eps_sb, EPS)
        # DoubleRow ones for the softmax denominator (DR psum outputs must
        # start at partition 0, so the denominator rides spare pa columns)
        ones8 = const.tile([P, 2, 32], fp8)
        nc.vector.memset(ones8, 1.0)

        bq_sb = const.tile([P, 4], fp32)
        nc.gpsimd.dma_start(out=bq_sb, in_=bq_d[:])
        bk_sb = const.tile([P, 4], fp32)
        nc.gpsimd.dma_start(out=bk_sb, in_=bk_d[:])
        bv_bc = const.tile([P, E], fp32)
        nc.gpsimd.dma_start(out=bv_bc, in_=bcast(bv_d[:]))
        b1_sb = const.tile([P, 12], fp32)
        nc.gpsimd.dma_start(out=b1_sb, in_=b1_d[:].rearrange("(c p) -> p c", p=P))
        b2_bc = const.tile([P, E], fp32)
        nc.gpsimd.dma_start(out=b2_bc, in_=bcast(b2_d[:]))

        # wpool tag "w": two slots rotating through wqkv -> xnT -> w1 -> w2
        # (xnT is dead after phase B+V)
        wqkv_sb = wpool.tile([P, 4, 3 * E], bf16, tag="w")
        nc.sync.dma_start(out=wqkv_sb, in_=wqkv_d[:].rearrange("(c p) n -> p c n", p=P))
        # LN1(x) transposed, token-block-major so each dma_start_transpose
        # writes a contiguous [P, 4, 128] destination
        xnT_sb = wpool.tile([P, 16, 4, P], bf16, tag="w")

        xq_sb = big.tile([P, 8, E], fp32)       # raw x query rows; becomes x2 in place
        qT_a = big.tile([P, 2, NQ], fp8)        # heads 0-3, partition 32g+c, slot s
        qT_b = big.tile([P, 2, NQ], fp8)        # heads 4-7
        kT_a = big.tile([P, 2, N], fp8)
        kT_b = big.tile([P, 2, N], fp8)
        v_sb = big.tile([P, 16, H, HD], fp8)    # token-major V
        att_sb = big.tile([P, 8, H, HD], bf16)
        xn2T_sb = big.tile([P, 4, NQ], bf16)
        g1r_sb = big.tile([P, 12, 512], bf16)   # fc1(qb=0) raw (bias added)
        g1T_sb = big.tile([P, 12, NQ], fp8)

        def ln_group(xts, xn_outs, batched=True, apply_eng=None):
            # LN over up to 4 token tiles; batched=True shares one Ln+Exp
            # pair across the group (fewer ACT instrs, +latency), while
            # batched=False pipelines per tile (for the latency-critical
            # tail). rstd via exp(-0.5*ln(var+eps)) keeps ACT on the
            # natural_log_exp table set (shared with softmax exp) — no
            # table switching against the attention exp stream.
            n = len(xts)
            groups = [range(n)] if batched else [[i] for i in range(n)]
            mv = work.tile([P, n, 2], fp32, tag="mv")
            for idxs in groups:
                for i in idxs:
                    stats = work.tile([P, 6], fp32, tag="st")
                    nc.vector.bn_stats(out=stats, in_=xts[i])
                    nc.vector.bn_aggr(out=mv[:, i, :], in_=stats)
                i0, ng = idxs[0], len(idxs)
                lnv = work.tile([P, ng], fp32, tag="lnv")
                nc.scalar.activation(
                    out=lnv, in_=mv[:, i0 : i0 + ng, 1], func=AF.Ln, bias=eps_sb, scale=1.0
                )
                rstd = work.tile([P, ng], fp32, tag="rstd")
                nc.scalar.activation(out=rstd, in_=lnv, func=AF.Exp, scale=-0.5)
                # NOTE: ACT Identity with a per-partition scale AP crashes the
                # device (NRT_EXEC_UNIT_UNRECOVERABLE). The apply runs on Pool
                # (SBUF-only op, Pool is otherwise idle) to keep DVE free.
                for j, i in enumerate(idxs):
                    (apply_eng or nc.gpsimd).tensor_scalar(
                        out=xn_outs[i], in0=xts[i],
                        scalar1=mv[:, i, 0:1], scalar2=rstd[:, j : j + 1],
                        op0=ALU.subtract, op1=ALU.mult,
                    )

        def transpose_to(dstT, xn, tok):
            # 4 PE transposes (bf16, 1 cyc/row) of one [128tok, 512E] tile into
            # one psum bank, then a single strided DVE copy (2x mode) into
            # dstT[:, :, tok*128:(tok+1)*128]; used for the latency-critical
            # LN2 path (phase A uses dma_start_transpose instead)
            pt = psum.tile([P, 512], bf16, tag="tp")
            for ec in range(4):
                nc.tensor.transpose(
                    pt[:, ec * P : (ec + 1) * P], xn[:, ec * P : (ec + 1) * P], id128b
                )
            nc.vector.tensor_copy(
                out=dstT[:, :, tok * P : (tok + 1) * P],
                in_=pt.rearrange("p (c t) -> p c t", c=4),
            )

        # ---------------- Phase A: load x, LN1, DMA-transpose to xnT ----------
        # x DMA dispatch alternates SP / ACT queues (both idle early); LN rstd
        # is batched 4 tiles per Ln+Exp pair; transposes run on the DMA
        # engines (dispatch: first half ACT, second half SP) so DVE only
        # carries bn_stats and the q/k/v psum evictions early on.
        def phase_a_group(g4):
            xts, xns = [], []
            for t in range(4 * g4, 4 * g4 + 4):
                if t < 8:
                    xt = xq_sb[:, t, :]
                else:
                    xt = xtp.tile([P, E], fp32, tag="xt", name=f"xt_{t}")
                eng = nc.scalar if t in (1, 3, 5, 7) else nc.sync
                eng.dma_start(out=xt, in_=x_view[t])
                xts.append(xt)
                xns.append(xtp.tile([P, E], bf16, tag="xn", name=f"xn_{t}"))
            ln_group(xts, xns)
            for i, xn in enumerate(xns):
                t = 4 * g4 + i
                if g4 == 0:
                    # group 0 is on the first-exp critical path: the PE+DVE
                    # transpose route is ~1.5us lower latency than the
                    # dma-transpose round trip
                    pt = psum.tile([P, 512], bf16, tag="tp")
                    for ec in range(4):
                        nc.tensor.transpose(
                            pt[:, ec * P : (ec + 1) * P],
                            xn[:, ec * P : (ec + 1) * P], id128b,
                        )
                    nc.vector.tensor_copy(
                        out=xnT_sb[:, t, :, :],
                        in_=pt.rearrange("p (c t) -> p c t", c=4),
                    )
                else:
                    teng = nc.sync
                    teng.dma_start_transpose(out=xnT_sb[:, t, :, :], in_=xn)

        phase_a_group(0)
        phase_a_group(1)

        # ---------------- Phase B: QKV matmuls (bf16) ----------------
        # PE issues its stream in order, so any multi-microsecond run of
        # bf16 matmuls starves the ACT exp spine. Only window 0 of K/Q is
        # emitted up front (enough for the first scores); everything else
        # becomes filler closures pumped one-per-kc inside scores blocks.
        kT_x = [kT_a, kT_b]
        qT_x = [qT_a, qT_b]

        def kq_group(w, m, is_k, evict_on_act=False):
            base = 512 if is_k else 0
            dst = (kT_x if is_k else qT_x)[m // 2]
            bias = (bk_sb if is_k else bq_sb)[:, m : m + 1]
            pt = psum.tile([P, 512], fp32, tag="tp")
            for ec in range(4):
                nc.tensor.matmul(
                    pt,
                    lhsT=wqkv_sb[:, ec, base + m * P : base + (m + 1) * P],
                    rhs=xnT_sb[:, 4 * w : 4 * w + 4, ec, :],
                    start=(ec == 0), stop=(ec == 3),
                )
            out = dst[:, m % 2, w * 512 : (w + 1) * 512]
            if evict_on_act:
                # prologue evictions ride the idle ACT engine (Identity is
                # resident in every activation table — no load)
                nc.scalar.activation(out=out, in_=pt, func=AF.Identity, bias=bias, scale=1.0)
            else:
                nc.vector.tensor_scalar_add(out=out, in0=pt, scalar1=bias)

        def v_group(tcn):
            pt = psum.tile([P, 512], fp32, tag="tp")
            for ec in range(4):
                nc.tensor.matmul(
                    pt,
                    lhsT=xnT_sb[:, tcn, ec, :],
                    rhs=wqkv_sb[:, ec, 1024:1536],
                    start=(ec == 0), stop=(ec == 3),
                )
            # the v bias rides the eviction (softmax rows sum to 1, so
            # attnV(v + bv) = attnV(v) + bv — no separate post-add needed)
            nc.vector.tensor_tensor(
                out=v_sb[:, tcn, :, :],
                in0=pt.rearrange("p (h d) -> p h d", h=H),
                in1=bv_bc.rearrange("p (h d) -> p h d", h=H),
                op=ALU.add,
            )

        from collections import deque

        fillers = deque()

        def pump(k=1):
            for _ in range(min(k, len(fillers))):
                fillers.popleft()()

        # ---------------- Phases C/D/E interleaved per query block ----------------
        w1_sb = wpool.tile([P, 4, HID], bf16, tag="w")
        nc.sync.dma_start(out=w1_sb, in_=w1_d[:].rearrange("(c p) n -> p c n", p=P))
        w2_sb = wpool.tile([P, 12, E], fp8, tag="w")
        nc.sync.dma_start(out=w2_sb, in_=w2_d[:].rearrange("(c p) n -> p c n", p=P))

        def scores_block(qb, jh, pump_from=0, pump_k=1):
            # halves[kh][:, kc8, h2, :] = exp(scores/8) fp8 for head 2*jh+h2,
            # keys (kh*4+kc8)*128..+128; pumps filler PE work after each kc
            # so the exp-paced stream never leaves PE with a long stall run
            kt, qt = kT_x[jh // 2], qT_x[jh // 2]
            g0 = 2 * (jh % 2)
            halves = []
            for kh in range(4):
                expSp = expp.tile([P, 4, 2, 512], fp8, tag="es")
                halves.append(expSp)
                for kc8 in range(4):
                    kc = kh * 4 + kc8
                    pt = psum.tile([P, 1024], fp32, tag="sc")
                    for h2 in range(2):
                        g = g0 + h2
                        nc.tensor.matmul(
                            pt[:, h2 * 512 : (h2 + 1) * 512],
                            lhsT=kt[32 * g : 32 * g + 32, :, kc * P : (kc + 1) * P],
                            rhs=qt[32 * g : 32 * g + 32, :, qb * 512 : (qb + 1) * 512],
                            start=True, stop=True, perf_mode=DR,
                            tile_position=(32 * g, 0),
                        )
                    nc.scalar.activation(
                        out=expSp[:, kc8, :, :], in_=pt, func=AF.Exp, scale=HD**-0.5
                    )
                    if kc >= pump_from:
                        pump(pump_k)
            return halves

        def attnv_parts(qb, jh, halves, h2):
            # split into two pumpable closures (~0.9us PE each) so a single
            # attnV burst never delays the next scores matmul by >1 exp
            h = 2 * jh + h2
            state = {}

            def emit_mm(pa, lo, hi):
                for kc2 in range(lo, hi):
                    kc = 2 * kc2
                    rhs = halves[kc // 4][:, kc % 4 : kc % 4 + 2, h2, :]
                    # attnV out in cols 0:512, softmax denominator (32
                    # identical rows from the ones stationary) in 512:1024
                    nc.tensor.matmul(
                        pa[:, 0:512],
                        lhsT=v_sb[:, kc : kc + 2, h, :], rhs=rhs,
                        start=(kc2 == 0), stop=(kc2 == 7), perf_mode=DR,
                    )
                    nc.tensor.matmul(
                        pa[0:32, 512:1024],
                        lhsT=ones8, rhs=rhs,
                        start=(kc2 == 0), stop=(kc2 == 7), perf_mode=DR,
                        skip_group_check=True,
                    )

            def mm1():
                pa = psum1.tile([64, 1024], fp32, tag="pa", name=f"pa_{qb}{jh}{h2}")
                state["pa"] = pa
                emit_mm(pa, 0, 4)

            def mm2():
                pa = state["pa"]
                emit_mm(pa, 4, 8)
                ah = work.tile([65, 512], fp32, tag="ah", name=f"ah_{qb}{jh}{h2}")
                nc.vector.tensor_copy(out=ah[0:64, :], in_=pa[:, 0:512])
                nc.vector.tensor_copy(out=ah[64:65, :], in_=pa[0:1, 512:1024])
                pt2 = psum.tile([P, 4, 65], fp32, tag="tp", name=f"pt2_{qb}{jh}{h2}")
                for qs in range(4):
                    nc.tensor.transpose(
                        pt2[:, qs, :], ah[:, qs * P : (qs + 1) * P],
                        id128[0:65, 0:65],
                    )
                for qs in range(4):
                    tcq = qb * 4 + qs
                    rec = work.tile([P, 1], fp32, tag="rec", name=f"rec_{qb}{jh}{h2}{qs}")
                    nc.vector.reciprocal(out=rec, in_=pt2[:, qs, 64:65])
                    nc.vector.tensor_scalar_mul(
                        out=att_sb[:, tcq, h, :], in0=pt2[:, qs, 0:HD], scalar1=rec
                    )

            return mm1, mm2

        def attnv_enqueue(qb, jh, halves):
            for h2 in range(2):
                mm1, mm2 = attnv_parts(qb, jh, halves, h2)
                fillers.append(mm1)
                fillers.append(mm2)

        def attnv_block(qb, jh, halves):
            for h2 in range(2):
                mm1, mm2 = attnv_parts(qb, jh, halves, h2)
                mm1()
                mm2()

        def residual_ln2_block(qb, batched=True):
            # the tail block (qb=1) keeps everything on DVE: cross-engine
            # sem hops cost more than DVE serialization on the critical tail
            tcns = list(range(qb * 4, qb * 4 + 4))
            add_eng = nc.gpsimd if batched else nc.vector
            xn2s = []
            for tcn in tcns:
                x2t = xq_sb[:, tcn, :]
                add_eng.tensor_tensor(
                    out=x2t, in0=x2t,
                    in1=att_sb[:, tcn].rearrange("p h d -> p (h d)"), op=ALU.add,
                )
                xn2s.append(xtp.tile([P, E], bf16, tag="xn", name=f"xn2_{tcn}"))
            ln_group(
                [xq_sb[:, tcn, :] for tcn in tcns], xn2s, batched=batched,
            )
            for tcn, xn2 in zip(tcns, xn2s):
                transpose_to(xn2T_sb, xn2, tcn)
                # pre-add the fc2 bias into the residual now (LN2 already
                # consumed x2), shortening the final eviction to one add
                nc.gpsimd.tensor_tensor(
                    out=xq_sb[:, tcn, :], in0=xq_sb[:, tcn, :], in1=b2_bc, op=ALU.add
                )

        def fc1_group(qb, mh):
            pt = psum.tile([P, 512], fp32, tag="tp")
            if qb == 0:
                for ec in range(4):
                    nc.tensor.matmul(
                        pt,
                        lhsT=w1_sb[:, ec, mh * P : (mh + 1) * P],
                        rhs=xn2T_sb[:, ec, 0:512],
                        start=(ec == 0), stop=(ec == 3),
                    )
            else:
                # tail half: token-tile column chunks so each matmul only
                # waits its own xn2T tile (the last LN2 tile gates 1/4 of
                # the psum instead of all of it)
                for tq in range(4):
                    cols = slice((4 + tq) * P, (5 + tq) * P)
                    for ec in range(4):
                        nc.tensor.matmul(
                            pt[:, tq * P : (tq + 1) * P],
                            lhsT=w1_sb[:, ec, mh * P : (mh + 1) * P],
                            rhs=xn2T_sb[:, ec, cols],
                            start=(ec == 0), stop=(ec == 3),
                        )
            if qb == 0:
                # raw bf16 evict with bias folded in (DVE); one wide gelu
                # later keeps ACT on the exp table during attention
                nc.vector.tensor_scalar_add(
                    out=g1r_sb[:, mh, :], in0=pt, scalar1=b1_sb[:, mh : mh + 1]
                )
            else:
                # tail half: gelu-fused ACT eviction (ACT is free then,
                # DVE is the tail straggler)
                nc.scalar.activation(
                    out=g1T_sb[:, mh, 512:1024], in_=pt,
                    func=AF.Gelu, bias=b1_sb[:, mh : mh + 1], scale=1.0,
                )

        def fc2_block(tcn):
            pt = psum.tile([P, 512], fp32, tag="tp")
            for jp in range(6):
                nc.tensor.matmul(
                    pt,
                    lhsT=g1T_sb[:, 2 * jp : 2 * jp + 2, tcn * P : (tcn + 1) * P],
                    rhs=w2_sb[:, 2 * jp : 2 * jp + 2, :],
                    start=(jp == 0), stop=(jp == 5), perf_mode=DR,
                )
            ot = work.tile([P, E], fp32, tag="ot")
            nc.vector.tensor_tensor(out=ot, in0=pt, in1=xq_sb[:, tcn, :], op=ALU.add)
            # tail tiles fan their DMA dispatch across the idle queues so the
            # last writeback isn't serialized behind seven others on SP
            eng = {4: nc.scalar, 5: nc.gpsimd, 6: nc.scalar, 7: nc.sync}.get(tcn, nc.sync)
            eng.dma_start(out=out_view[tcn], in_=ot)

        # ---- emission order IS the per-engine schedule (in-order issue) ----
        # Up-front PE work: only the quartet-a slots of K/Q window 0 — the
        # minimum for the first scores block. Everything else is drip-fed.
        for m in (0, 1):
            kq_group(0, m, is_k=True, evict_on_act=True)
        for m in (0, 1):
            kq_group(0, m, is_k=False, evict_on_act=True)
        phase_a_group(2)
        phase_a_group(3)
        # Filler order = earliest consumer: quartet-a K windows (blocks
        # (0,0)/(0,1)), then quartet-b K/Q w0 (blocks (0,2)/(0,3)), then Q
        # window 1 (qb=1 blocks). One closure per kc ≈ one exp of ACT time.
        for w in range(1, 4):
            for m in (0, 1):
                fillers.append(
                    lambda w=w, m=m: kq_group(w, m, is_k=True, evict_on_act=(w == 1))
                )
        for m in (2, 3):
            fillers.append(lambda m=m: kq_group(0, m, is_k=True))
            fillers.append(lambda m=m: kq_group(0, m, is_k=False))
        for w in range(1, 4):
            for m in (2, 3):
                fillers.append(lambda w=w, m=m: kq_group(w, m, is_k=True))
        for m in range(4):
            fillers.append(lambda m=m: kq_group(1, m, is_k=False))
        h00 = scores_block(0, 0, pump_from=0)
        for tcn in range(16):
            fillers.append(lambda tcn=tcn: v_group(tcn))
        h01 = scores_block(0, 1)
        attnv_enqueue(0, 0, h00)
        attnv_enqueue(0, 1, h01)
        h02 = scores_block(0, 2)
        attnv_enqueue(0, 2, h02)
        h03 = scores_block(0, 3)
        attnv_enqueue(0, 3, h03)
        h10 = scores_block(1, 0, pump_from=1)
        residual_ln2_block(0)  # after sc(1,0): its Ln/Exp never stalls ACT
        fillers.extend(
            lambda mh=mh: fc1_group(0, mh) for mh in range(12)
        )
        attnv_enqueue(1, 0, h10)
        h11 = scores_block(1, 1, pump_from=4)
        attnv_enqueue(1, 1, h11)
        h12 = scores_block(1, 2, pump_from=1)
        attnv_enqueue(1, 2, h12)
        h13 = scores_block(1, 3, pump_from=1)
        pump(len(fillers))

        # Gate the wide gelu on the LAST exp tile (bypass: data unchanged):
        # ungated, the scheduler hoists it mid-spine (g1r has been ready
        # since fc1(0)) and stretches the exp stream by ~8us + 2 table loads.
        probe = work.tile([P, 1], fp32, tag="probe")
        nc.gpsimd.tensor_copy(out=probe, in_=h13[3][:, 3, 1, 0:1])
        nc.gpsimd.tensor_scalar(
            out=g1r_sb[:, 0, 0:1], in0=g1r_sb[:, 0, 0:1], scalar1=probe,
            scalar2=None, op0=ALU.bypass,
        )
        nc.scalar.activation(
            out=g1T_sb[:, :, 0:512], in_=g1r_sb, func=AF.Gelu, scale=1.0
        )
        attnv_block(1, 3, h13)
        # fc2 for the gelu0 half fills PE while the residual(1) chain runs
        for tcn in range(0, 4):
            fc2_block(tcn)
        residual_ln2_block(1, batched=False)

        # tail fc2 rides the fused-gelu stream: each jp pair of gelus
        # immediately feeds its 4 accumulation matmuls (psum pairs in the
        # now-free sc banks), so the last gelu only gates one jp round
        pts = [
            psum.tile([P, 1024], fp32, tag="sc", name="fc2_45"),
            psum.tile([P, 1024], fp32, tag="sc", name="fc2_67"),
        ]
        for mh in range(12):
            fc1_group(1, mh)
            if mh % 2 == 1:
                jp = mh // 2
                for tcn in range(4, 8):
                    nc.tensor.matmul(
                        pts[(tcn - 4) // 2][:, (tcn % 2) * 512 : (tcn % 2 + 1) * 512],
                        lhsT=g1T_sb[:, 2 * jp : 2 * jp + 2, tcn * P : (tcn + 1) * P],
                        rhs=w2_sb[:, 2 * jp : 2 * jp + 2, :],
                        start=(jp == 0), stop=(jp == 5), perf_mode=DR,
                        skip_group_check=True,
                    )
        for tcn in range(4, 8):
            ot = work.tile([P, E], fp32, tag="ot", name=f"ot_{tcn}")
            nc.vector.tensor_tensor(
                out=ot,
                in0=pts[(tcn - 4) // 2][:, (tcn % 2) * 512 : (tcn % 2 + 1) * 512],
                in1=xq_sb[:, tcn, :], op=ALU.add,
            )
            eng = {4: nc.scalar, 5: nc.gpsimd, 6: nc.scalar, 7: nc.sync}.get(tcn)
            eng.dma_start(out=out_view[tcn], in_=ot)

    if split_waits:
        _split_matmul_waits(nc, mybir)
    return nc


def _split_matmul_waits(nc, mybir):
    """walrus allows only one sync wait per engine instruction; hoist extra
    waits onto same-engine NoOps placed just before (NX dispatch is in-order,
    so the nops' waits gate the instruction)."""
    k = 0
    for fn in nc.m.functions:
        for blk in fn.blocks:
            new = []
            for inst in blk.instructions:
                si = inst.sync_info
                if si is not None and si.on_wait and len(si.on_wait) > 1:
                    for w in si.on_wait[:-1]:
                        nop = mybir.InstNoOp(name=f"waitnop-{k}", ins=[], outs=[])
                        k += 1
                        nop.engine = inst.engine
                        nop.sync_info = mybir.SyncInfo(on_wait=[w], on_update=[])
                        new.append(nop)
                    inst.sync_info = mybir.SyncInfo(
                        on_wait=[si.on_wait[-1]], on_update=si.on_update
                    )
                new.append(inst)
            blk.instructions[:] = new


def _get_nc():
    if "nc" not in _NC_CACHE:
        _NC_CACHE["nc"] = _build_nc()
    return _NC_CACHE["nc"]


def _qk_slot_perm():
    # perm[j]: source channel (within a heads-major 512-col q or k section)
    # for permuted column j = m*128 + p, m = (quartet x)*2 + slot s; the
    # psum partition p then lands head 4x + p//32, channel 32s + p%32.
    perm = np.empty(512, np.int64)
    for m in range(4):
        x, s = m // 2, m % 2
        for p in range(P):
            perm[m * 128 + p] = (4 * x + p // 32) * 64 + s * 32 + (p % 32)
    return perm


def _prep_inputs(inputs):
    fp8 = ml_dtypes.float8_e4m3
    bf16 = ml_dtypes.bfloat16
    x = np.asarray(inputs["x"], np.float32)
    qkv_w = np.asarray(inputs["qkv_w"], np.float32)
    qkv_b = np.asarray(inputs["qkv_b"], np.float32)
    fc1_w = np.asarray(inputs["fc1_w"], np.float32)
    fc1_b = np.asarray(inputs["fc1_b"], np.float32)
    fc2_w = np.asarray(inputs["fc2_w"], np.float32)
    fc2_b = np.asarray(inputs["fc2_b"], np.float32)

    # reorder qkv channels: per-head interleave [q|k|v]*H -> heads-major
    # [Q|K|V], then slot-permute Q and K columns for the DoubleRow layout
    w3 = qkv_w.reshape(E, H, 3, HD)
    wq, wk, wv = (w3[:, :, i, :].reshape(E, E) for i in range(3))
    b3 = qkv_b.reshape(H, 3, HD)
    bq, bk, bv = (np.ascontiguousarray(b3[:, i, :].reshape(E)) for i in range(3))

    perm = _qk_slot_perm()
    wqkv = np.ascontiguousarray(
        np.concatenate([wq[:, perm], wk[:, perm], wv], axis=1)
    ).astype(bf16)
    bq_t = np.ascontiguousarray(bq[perm].reshape(4, P).T)  # [P, 4]
    bk_t = np.ascontiguousarray(bk[perm].reshape(4, P).T)

    w1 = np.ascontiguousarray(fc1_w).astype(bf16)
    w2 = np.ascontiguousarray(fc2_w).astype(fp8)

    in_maps = []
    for c in range(NCORES):
        b, half = c // 2, c % 2
        xr = np.ascontiguousarray(np.roll(x[b], -half * NQ, axis=0))
        in_maps.append(
            {
                "x": xr,
                "wqkv": wqkv,
                "bq": bq_t,
                "bk": bk_t,
                "bv": bv,
                "w1": w1,
                "b1": fc1_b,
                "w2": w2,
                "b2": fc2_b,
            }
        )
    return in_maps


def kernel(**inputs) -> np.ndarray:
    from concourse.bass_utils import run_bass_kernel_spmd

    nc = _get_nc()
    in_maps = _prep_inputs(inputs)
    res = run_bass_kernel_spmd(nc, in_maps, core_ids=list(range(NCORES)))
    y = np.empty((B, N, E), np.float32)
    for c in range(NCORES):
        b, half = c // 2, c % 2
        y[b, half * NQ : (half + 1) * NQ] = np.asarray(res.results[c]["out"])
    return y


if __name__ == "__main__":
    nc = _build_nc()
    print("build OK")


# revision 66
# speedup vs baseline: 1.3264x; 1.0014x over previous
"""Trainium2 Bass kernel for nn_EncoderLayer (B=4, N=2048, E=512, H=8, HIDDEN=1536).

Sharding: 8 cores; core c handles batch b=c//2, query-half c%2 (1024 query
rows). Each core computes K/V over the full 2048-row sequence of its batch
(keys are permutation-invariant under softmax, so the host rotates x[b] to put
the query rows first), and the FFN over its 1024 rows only.

Precision (driven by an error-attribution study against the fp32 reference):
attention (q/k/v, exp(scores), attnV) and fc2 run in fp8e4m3 DoubleRow perf
mode (2 reduction rows/partition, 0.5 cyc/row) — their quantization noise
washes out against the residual path. QKV projection and fc1 stay bf16 (fc1
error feeds gelu+fc2 directly and dominated the budget). Residual/LN fp32.

Layouts: per-head qk contraction ch -> (partition 32*g + ch%32, slot ch//32),
head quartets a=(0..3)/b=(4..7) stacked along partitions (host permutes wqkv
q/k columns so PSUM evictions land partition-aligned). fc2 contraction
hc -> (pair jp = hc//256, slot (hc//128)%2, p = hc%128) falls out of the
[P, 12, NQ] g1 layout for free.

Engine split: PE matmuls + attention-out transposes; ACT exp spine + gelu
(deferred/fused to bracket one table switch) + LN rstd via Ln/Exp (exp-table
resident); DVE LN stats, psum evictions, softmax scale; Pool (idle otherwise)
LN applies and residual adds; DMA engines do the LN1 transposes
(dma_start_transpose), keeping DVE off the phase-A critical path.
"""

import sys

sys.path.insert(0, "/opt/trn_rl_repo")

import numpy as np
import ml_dtypes

B, N, E = 4, 2048, 512
H, HD = 8, 64
HID = 3 * E
NQ = 1024  # query rows per core
P = 128
EPS = 1e-5
NCORES = 8

_NC_CACHE = {}


def _build_nc(split_waits=True):
    from contextlib import ExitStack

    import concourse.bass as bass
    import concourse.mybir as mybir
    import concourse.tile as tile
    from concourse.masks import make_identity

    fp32 = mybir.dt.float32
    bf16 = mybir.dt.bfloat16
    fp8 = mybir.dt.float8e4
    AF = mybir.ActivationFunctionType
    ALU = mybir.AluOpType
    DR = mybir.MatmulPerfMode.DoubleRow

    nc = bass.Bass()

    x_d = nc.declare_dram_parameter("x", [N, E], fp32, isOutput=False)
    wqkv_d = nc.declare_dram_parameter("wqkv", [E, 3 * E], bf16, isOutput=False)
    wqkv8_d = nc.declare_dram_parameter("wqkv8", [E, 3 * E], fp8, isOutput=False)
    bq_d = nc.declare_dram_parameter("bq", [P, 4], fp32, isOutput=False)
    bk_d = nc.declare_dram_parameter("bk", [P, 4], fp32, isOutput=False)
    bv_d = nc.declare_dram_parameter("bv", [E], fp32, isOutput=False)
    w1_d = nc.declare_dram_parameter("w1", [E, HID], bf16, isOutput=False)
    b1_d = nc.declare_dram_parameter("b1", [HID], fp32, isOutput=False)
    w2_d = nc.declare_dram_parameter("w2", [HID, E], fp8, isOutput=False)
    b2_d = nc.declare_dram_parameter("b2", [E], fp32, isOutput=False)
    out_d = nc.declare_dram_parameter("out", [NQ, E], fp32, isOutput=True)

    x_view = x_d[:].rearrange("(t p) e -> t p e", p=P)  # [16, 128, 512]
    out_view = out_d[:].rearrange("(t p) e -> t p e", p=P)  # [8, 128, 512]

    def bcast(ap, parts=P):
        return bass.AP(tensor=ap.tensor, offset=ap.offset, ap=[[0, parts]] + list(ap.ap))

    with tile.TileContext(nc) as tc, ExitStack() as ctx:
        const = ctx.enter_context(tc.tile_pool(name="const", bufs=1))
        big = ctx.enter_context(tc.tile_pool(name="big", bufs=1))
        wpool = ctx.enter_context(tc.tile_pool(name="wpool", bufs=2))
        work = ctx.enter_context(tc.tile_pool(name="work", bufs=3))
        xtp = ctx.enter_context(tc.tile_pool(name="xtp", bufs=4))
        expp = ctx.enter_context(tc.tile_pool(name="expp", bufs=12))
        psum = ctx.enter_context(tc.tile_pool(name="psum", bufs=2, space="PSUM"))
        psum1 = ctx.enter_context(tc.tile_pool(name="psum1", bufs=1, space="PSUM"))

        id128 = const.tile([P, P], fp32)
        make_identity(nc, id128)
        id128b = const.tile([P, P], bf16)
        nc.vector.tensor_copy(out=id128b, in_=id128)
        eps_sb = const.tile([P, 1], fp32)
        nc.vector.memset(eps_sb, EPS)
        # DoubleRow ones for the softmax denominator (DR psum outputs must
        # start at partition 0, so the denominator rides spare pa columns)
        ones8 = const.tile([P, 2, 32], fp8)
        nc.vector.memset(ones8, 1.0)

        bq_sb = const.tile([P, 4], fp32)
        bk_sb = const.tile([P, 4], fp32)
        bv_bc = const.tile([P, E], fp32)
        b1_sb = const.tile([P, 12], fp32)
        b2_bc = const.tile([P, E], fp32)

        # wpool tag "w": two slots rotating through wqkv -> xnT -> w1 -> w2
        # (xnT is dead after phase B+V)
        wqkv_sb = wpool.tile([P, 4, 3 * E], bf16, tag="w")
        nc.sync.dma_start(out=wqkv_sb, in_=wqkv_d[:].rearrange("(c p) n -> p c n", p=P))
        # LN1(x) transposed, token-block-major so each dma_start_transpose
        # writes a contiguous [P, 4, 128] destination
        xnT_sb = wpool.tile([P, 16, 4, P], bf16, tag="w")

        wqkv8_sb = big.tile([P, 4, 3 * E], fp8)
        nc.sync.dma_start(out=wqkv8_sb, in_=wqkv8_d[:].rearrange("(c p) n -> p c n", p=P))
        xnT8_sb = big.tile([P, 16, 4, P], fp8)  # Pool-copied fp8 view of xnT
        xq_sb = big.tile([P, 8, E], fp32)       # raw x query rows; becomes x2 in place
        qT_a = big.tile([P, 2, NQ], fp8)        # heads 0-3, partition 32g+c, slot s
        qT_b = big.tile([P, 2, NQ], fp8)        # heads 4-7
        kT_a = big.tile([P, 2, N], fp8)
        kT_b = big.tile([P, 2, N], fp8)
        v_sb = big.tile([P, 16, H, HD], fp8)    # token-major V
        att_sb = big.tile([P, 8, H, HD], bf16)
        xn2T_sb = big.tile([P, 4, NQ], bf16)
        g1r_sb = big.tile([P, 12, 512], bf16)   # fc1(qb=0) raw (bias added)
        g1T_sb = big.tile([P, 12, NQ], fp8)

        def ln_group(xts, xn_outs, batched=True, apply_eng=None):
            # LN over up to 4 token tiles; batched=True shares one Ln+Exp
            # pair across the group (fewer ACT instrs, +latency), while
            # batched=False pipelines per tile (for the latency-critical
            # tail). rstd via exp(-0.5*ln(var+eps)) keeps ACT on the
            # natural_log_exp table set (shared with softmax exp) — no
            # table switching against the attention exp stream.
            n = len(xts)
            groups = [range(n)] if batched else [[i] for i in range(n)]
            mv = work.tile([P, n, 2], fp32, tag="mv")
            for idxs in groups:
                for i in idxs:
                    stats = work.tile([P, 6], fp32, tag="st")
                    nc.vector.bn_stats(out=stats, in_=xts[i])
                    nc.vector.bn_aggr(out=mv[:, i, :], in_=stats)
                i0, ng = idxs[0], len(idxs)
                lnv = work.tile([P, ng], fp32, tag="lnv")
                nc.scalar.activation(
                    out=lnv, in_=mv[:, i0 : i0 + ng, 1], func=AF.Ln, bias=eps_sb, scale=1.0
                )
                rstd = work.tile([P, ng], fp32, tag="rstd")
                nc.scalar.activation(out=rstd, in_=lnv, func=AF.Exp, scale=-0.5)
                # NOTE: ACT Identity with a per-partition scale AP crashes the
                # device (NRT_EXEC_UNIT_UNRECOVERABLE). The apply runs on Pool
                # (SBUF-only op, Pool is otherwise idle) to keep DVE free.
                for j, i in enumerate(idxs):
                    (apply_eng or nc.gpsimd).tensor_scalar(
                        out=xn_outs[i], in0=xts[i],
                        scalar1=mv[:, i, 0:1], scalar2=rstd[:, j : j + 1],
                        op0=ALU.subtract, op1=ALU.mult,
                    )

        def transpose_to(dstT, xn, tok):
            # 4 PE transposes (bf16, 1 cyc/row) of one [128tok, 512E] tile into
            # one psum bank, then a single strided DVE copy (2x mode) into
            # dstT[:, :, tok*128:(tok+1)*128]; used for the latency-critical
            # LN2 path (phase A uses dma_start_transpose instead)
            pt = psum.tile([P, 512], bf16, tag="tp")
            for ec in range(4):
                nc.tensor.transpose(
                    pt[:, ec * P : (ec + 1) * P], xn[:, ec * P : (ec + 1) * P], id128b
                )
            nc.vector.tensor_copy(
                out=dstT[:, :, tok * P : (tok + 1) * P],
                in_=pt.rearrange("p (c t) -> p c t", c=4),
            )

        # ---------------- Phase A: load x, LN1, DMA-transpose to xnT ----------
        # x DMA dispatch alternates SP / ACT queues (both idle early); LN rstd
        # is batched 4 tiles per Ln+Exp pair; transposes run on the DMA
        # engines (dispatch: first half ACT, second half SP) so DVE only
        # carries bn_stats and the q/k/v psum evictions early on.
        def phase_a_group(g4):
            xts, xns = [], []
            for t in range(4 * g4, 4 * g4 + 4):
                if t < 8:
                    xt = xq_sb[:, t, :]
                else:
                    xt = xtp.tile([P, E], fp32, tag="xt", name=f"xt_{t}")
                eng = nc.scalar if t in (1, 3, 5, 7) else nc.sync
                eng.dma_start(out=xt, in_=x_view[t])
                xts.append(xt)
                xns.append(xtp.tile([P, E], bf16, tag="xn", name=f"xn_{t}"))
            ln_group(xts, xns)
            for i, xn in enumerate(xns):
                t = 4 * g4 + i
                if g4 == 0:
                    # group 0 is on the first-exp critical path: the PE+DVE
                    # transpose route is ~1.5us lower latency than the
                    # dma-transpose round trip
                    pt = psum.tile([P, 512], bf16, tag="tp")
                    for ec in range(4):
                        nc.tensor.transpose(
                            pt[:, ec * P : (ec + 1) * P],
                            xn[:, ec * P : (ec + 1) * P], id128b,
                        )
                    nc.vector.tensor_copy(
                        out=xnT_sb[:, t, :, :],
                        in_=pt.rearrange("p (c t) -> p c t", c=4),
                    )
                else:
                    teng = nc.sync
                    teng.dma_start_transpose(out=xnT_sb[:, t, :, :], in_=xn)
            nc.gpsimd.tensor_copy(
                out=xnT8_sb[:, 4 * g4 : 4 * g4 + 4, :, :],
                in_=xnT_sb[:, 4 * g4 : 4 * g4 + 4, :, :],
            )

        phase_a_group(0)
        phase_a_group(1)
        # bias loads ride the Pool SWDGE queue *behind* the group-0/1 LN
        # applies so they never delay the first-exp chain
        nc.gpsimd.dma_start(out=bq_sb, in_=bq_d[:])
        nc.gpsimd.dma_start(out=bk_sb, in_=bk_d[:])
        nc.gpsimd.dma_start(out=bv_bc, in_=bcast(bv_d[:]))
        nc.gpsimd.dma_start(out=b1_sb, in_=b1_d[:].rearrange("(c p) -> p c", p=P))
        nc.gpsimd.dma_start(out=b2_bc, in_=bcast(b2_d[:]))

        # ---------------- Phase B: QKV matmuls (bf16) ----------------
        # PE issues its stream in order, so any multi-microsecond run of
        # bf16 matmuls starves the ACT exp spine. Only window 0 of K/Q is
        # emitted up front (enough for the first scores); everything else
        # becomes filler closures pumped one-per-kc inside scores blocks.
        kT_x = [kT_a, kT_b]
        qT_x = [qT_a, qT_b]

        def kq_group(w, m, is_k, evict_on_act=False):
            base = 512 if is_k else 0
            dst = (kT_x if is_k else qT_x)[m // 2]
            bias = (bk_sb if is_k else bq_sb)[:, m : m + 1]
            pt = psum.tile([P, 512], fp32, tag="tp")
            for ec in range(4):
                nc.tensor.matmul(
                    pt,
                    lhsT=wqkv_sb[:, ec, base + m * P : base + (m + 1) * P],
                    rhs=xnT_sb[:, 4 * w : 4 * w + 4, ec, :],
                    start=(ec == 0), stop=(ec == 3),
                )
            out = dst[:, m % 2, w * 512 : (w + 1) * 512]
            if evict_on_act:
                # prologue evictions ride the idle ACT engine (Identity is
                # resident in every activation table — no load)
                nc.scalar.activation(out=out, in_=pt, func=AF.Identity, bias=bias, scale=1.0)
            else:
                nc.vector.tensor_scalar_add(out=out, in0=pt, scalar1=bias)

        def v_group(tcn):
            pt = psum.tile([P, 512], fp32, tag="tp")
            for ecp in range(2):
                nc.tensor.matmul(
                    pt,
                    lhsT=xnT8_sb[:, tcn, 2 * ecp : 2 * ecp + 2, :],
                    rhs=wqkv8_sb[:, 2 * ecp : 2 * ecp + 2, 1024:1536],
                    start=(ecp == 0), stop=(ecp == 1), perf_mode=DR,
                )
            # the v bias rides the eviction (softmax rows sum to 1, so
            # attnV(v + bv) = attnV(v) + bv — no separate post-add needed)
            nc.vector.tensor_tensor(
                out=v_sb[:, tcn, :, :],
                in0=pt.rearrange("p (h d) -> p h d", h=H),
                in1=bv_bc.rearrange("p (h d) -> p h d", h=H),
                op=ALU.add,
            )

        from collections import deque

        fillers = deque()

        def pump(k=1):
            for _ in range(min(k, len(fillers))):
                fillers.popleft()()

        # ---------------- Phases C/D/E interleaved per query block ----------------
        w1_sb = wpool.tile([P, 4, HID], bf16, tag="w")
        nc.sync.dma_start(out=w1_sb, in_=w1_d[:].rearrange("(c p) n -> p c n", p=P))
        w2_sb = wpool.tile([P, 12, E], fp8, tag="w")
        nc.sync.dma_start(out=w2_sb, in_=w2_d[:].rearrange("(c p) n -> p c n", p=P))

        def scores_block(qb, jh, pump_from=0, pump_k=1):
            # halves[kh][:, kc8, h2, :] = exp(scores/8) fp8 for head 2*jh+h2,
            # keys (kh*4+kc8)*128..+128; pumps filler PE work after each kc
            # so the exp-paced stream never leaves PE with a long stall run
            kt, qt = kT_x[jh // 2], qT_x[jh // 2]
            g0 = 2 * (jh % 2)
            halves = []
            for kh in range(4):
                expSp = expp.tile([P, 4, 2, 512], fp8, tag="es")
                halves.append(expSp)
                for kc8 in range(4):
                    kc = kh * 4 + kc8
                    pt = psum.tile([P, 1024], fp32, tag="sc")
                    for h2 in range(2):
                        g = g0 + h2
                        nc.tensor.matmul(
                            pt[:, h2 * 512 : (h2 + 1) * 512],
                            lhsT=kt[32 * g : 32 * g + 32, :, kc * P : (kc + 1) * P],
                            rhs=qt[32 * g : 32 * g + 32, :, qb * 512 : (qb + 1) * 512],
                            start=True, stop=True, perf_mode=DR,
                            tile_position=(32 * g, 0),
                        )
                    nc.scalar.activation(
                        out=expSp[:, kc8, :, :], in_=pt, func=AF.Exp, scale=HD**-0.5
                    )
                    if kc >= pump_from:
                        pump(pump_k)
            return halves

        def attnv_parts(qb, jh, halves, h2):
            # split into two pumpable closures (~0.9us PE each) so a single
            # attnV burst never delays the next scores matmul by >1 exp
            h = 2 * jh + h2
            state = {}

            def emit_mm(pa, lo, hi):
                for kc2 in range(lo, hi):
                    kc = 2 * kc2
                    rhs = halves[kc // 4][:, kc % 4 : kc % 4 + 2, h2, :]
                    # attnV out in cols 0:512, softmax denominator (32
                    # identical rows from the ones stationary) in 512:1024
                    nc.tensor.matmul(
                        pa[:, 0:512],
                        lhsT=v_sb[:, kc : kc + 2, h, :], rhs=rhs,
                        start=(kc2 == 0), stop=(kc2 == 7), perf_mode=DR,
                    )
                    nc.tensor.matmul(
                        pa[0:32, 512:1024],
                        lhsT=ones8, rhs=rhs,
                        start=(kc2 == 0), stop=(kc2 == 7), perf_mode=DR,
                        skip_group_check=True,
                    )

            def mm1():
                pa = psum1.tile([64, 1024], fp32, tag="pa", name=f"pa_{qb}{jh}{h2}")
                state["pa"] = pa
                emit_mm(pa, 0, 4)

            def mm2():
                pa = state["pa"]
                emit_mm(pa, 4, 8)
                ah = work.tile([65, 512], fp32, tag="ah", name=f"ah_{qb}{jh}{h2}")
                nc.vector.tensor_copy(out=ah[0:64, :], in_=pa[:, 0:512])
                nc.vector.tensor_copy(out=ah[64:65, :], in_=pa[0:1, 512:1024])
                pt2 = psum.tile([P, 4, 65], fp32, tag="tp", name=f"pt2_{qb}{jh}{h2}")
                for qs in range(4):
                    nc.tensor.transpose(
                        pt2[:, qs, :], ah[:, qs * P : (qs + 1) * P],
                        id128[0:65, 0:65],
                    )
                for qs in range(4):
                    tcq = qb * 4 + qs
                    rec = work.tile([P, 1], fp32, tag="rec", name=f"rec_{qb}{jh}{h2}{qs}")
                    nc.vector.reciprocal(out=rec, in_=pt2[:, qs, 64:65])
                    nc.vector.tensor_scalar_mul(
                        out=att_sb[:, tcq, h, :], in0=pt2[:, qs, 0:HD], scalar1=rec
                    )

            return mm1, mm2

        def attnv_enqueue(qb, jh, halves):
            for h2 in range(2):
                mm1, mm2 = attnv_parts(qb, jh, halves, h2)
                fillers.append(mm1)
                fillers.append(mm2)

        def attnv_block(qb, jh, halves):
            for h2 in range(2):
                mm1, mm2 = attnv_parts(qb, jh, halves, h2)
                mm1()
                mm2()

        def residual_ln2_block(qb, batched=True):
            # the tail block (qb=1) keeps everything on DVE: cross-engine
            # sem hops cost more than DVE serialization on the critical tail
            tcns = list(range(qb * 4, qb * 4 + 4))
            add_eng = nc.gpsimd if batched else nc.vector
            xn2s = []
            for tcn in tcns:
                x2t = xq_sb[:, tcn, :]
                add_eng.tensor_tensor(
                    out=x2t, in0=x2t,
                    in1=att_sb[:, tcn].rearrange("p h d -> p (h d)"), op=ALU.add,
                )
                xn2s.append(xtp.tile([P, E], bf16, tag="xn", name=f"xn2_{tcn}"))
            ln_group(
                [xq_sb[:, tcn, :] for tcn in tcns], xn2s, batched=batched,
            )
            for tcn, xn2 in zip(tcns, xn2s):
                transpose_to(xn2T_sb, xn2, tcn)
                # pre-add the fc2 bias into the residual now (LN2 already
                # consumed x2), shortening the final eviction to one add
                nc.gpsimd.tensor_tensor(
                    out=xq_sb[:, tcn, :], in0=xq_sb[:, tcn, :], in1=b2_bc, op=ALU.add
                )

        def fc1_group(qb, mh):
            pt = psum.tile([P, 512], fp32, tag="tp")
            if qb == 0:
                for ec in range(4):
                    nc.tensor.matmul(
                        pt,
                        lhsT=w1_sb[:, ec, mh * P : (mh + 1) * P],
                        rhs=xn2T_sb[:, ec, 0:512],
                        start=(ec == 0), stop=(ec == 3),
                    )
            else:
                # tail half: token-tile column chunks so each matmul only
                # waits its own xn2T tile (the last LN2 tile gates 1/4 of
                # the psum instead of all of it)
                for tq in range(4):
                    cols = slice((4 + tq) * P, (5 + tq) * P)
                    for ec in range(4):
                        nc.tensor.matmul(
                            pt[:, tq * P : (tq + 1) * P],
                            lhsT=w1_sb[:, ec, mh * P : (mh + 1) * P],
                            rhs=xn2T_sb[:, ec, cols],
                            start=(ec == 0), stop=(ec == 3),
                        )
            if qb == 0:
                # raw bf16 evict with bias folded in (DVE); one wide gelu
                # later keeps ACT on the exp table during attention
                nc.vector.tensor_scalar_add(
                    out=g1r_sb[:, mh, :], in0=pt, scalar1=b1_sb[:, mh : mh + 1]
                )
            else:
                # tail half: gelu-fused ACT eviction (ACT is free then,
                # DVE is the tail straggler)
                nc.scalar.activation(
                    out=g1T_sb[:, mh, 512:1024], in_=pt,
                    func=AF.Gelu, bias=b1_sb[:, mh : mh + 1], scale=1.0,
                )

        def fc2_block(tcn):
            pt = psum.tile([P, 512], fp32, tag="tp")
            for jp in range(6):
                nc.tensor.matmul(
                    pt,
                    lhsT=g1T_sb[:, 2 * jp : 2 * jp + 2, tcn * P : (tcn + 1) * P],
                    rhs=w2_sb[:, 2 * jp : 2 * jp + 2, :],
                    start=(jp == 0), stop=(jp == 5), perf_mode=DR,
                )
            ot = work.tile([P, E], fp32, tag="ot")
            nc.vector.tensor_tensor(out=ot, in0=pt, in1=xq_sb[:, tcn, :], op=ALU.add)
            # tail tiles fan their DMA dispatch across the idle queues so the
            # last writeback isn't serialized behind seven others on SP
            eng = {4: nc.scalar, 5: nc.gpsimd, 6: nc.scalar, 7: nc.sync}.get(tcn, nc.sync)
            eng.dma_start(out=out_view[tcn], in_=ot)

        # ---- emission order IS the per-engine schedule (in-order issue) ----
        # Up-front PE work: only the quartet-a slots of K/Q window 0 — the
        # minimum for the first scores block. Everything else is drip-fed.
        for m in (0, 1):
            kq_group(0, m, is_k=True, evict_on_act=True)
        for m in (0, 1):
            kq_group(0, m, is_k=False, evict_on_act=True)
        phase_a_group(2)
        phase_a_group(3)
        # Filler order = earliest consumer: quartet-a K windows (blocks
        # (0,0)/(0,1)), then quartet-b K/Q w0 (blocks (0,2)/(0,3)), then Q
        # window 1 (qb=1 blocks). One closure per kc ≈ one exp of ACT time.
        for w in range(1, 4):
            for m in (0, 1):
                fillers.append(
                    lambda w=w, m=m: kq_group(w, m, is_k=True, evict_on_act=(w == 1))
                )
        for m in (2, 3):
            fillers.append(lambda m=m: kq_group(0, m, is_k=True))
            fillers.append(lambda m=m: kq_group(0, m, is_k=False))
        for w in range(1, 4):
            for m in (2, 3):
                fillers.append(lambda w=w, m=m: kq_group(w, m, is_k=True))
        for m in range(4):
            fillers.append(lambda m=m: kq_group(1, m, is_k=False))
        h00 = scores_block(0, 0, pump_from=0)
        for tcn in range(16):
            fillers.append(lambda tcn=tcn: v_group(tcn))
        h01 = scores_block(0, 1)
        attnv_enqueue(0, 0, h00)
        attnv_enqueue(0, 1, h01)
        h02 = scores_block(0, 2)
        attnv_enqueue(0, 2, h02)
        h03 = scores_block(0, 3)
        attnv_enqueue(0, 3, h03)
        h10 = scores_block(1, 0, pump_from=1)
        residual_ln2_block(0)  # after sc(1,0): its Ln/Exp never stalls ACT
        fillers.extend(
            lambda mh=mh: fc1_group(0, mh) for mh in range(12)
        )
        attnv_enqueue(1, 0, h10)
        h11 = scores_block(1, 1, pump_from=4)
        attnv_enqueue(1, 1, h11)
        h12 = scores_block(1, 2, pump_from=1)
        attnv_enqueue(1, 2, h12)
        h13 = scores_block(1, 3, pump_from=1)
        pump(len(fillers))

        # Gate the wide gelu on the LAST exp tile (bypass: data unchanged):
        # ungated, the scheduler hoists it mid-spine (g1r has been ready
        # since fc1(0)) and stretches the exp stream by ~8us + 2 table loads.
        probe = work.tile([P, 1], fp32, tag="probe")
        nc.gpsimd.tensor_copy(out=probe, in_=h13[3][:, 3, 1, 0:1])
        nc.gpsimd.tensor_scalar(
            out=g1r_sb[:, 0, 0:1], in0=g1r_sb[:, 0, 0:1], scalar1=probe,
            scalar2=None, op0=ALU.bypass,
        )
        nc.scalar.activation(
            out=g1T_sb[:, :, 0:512], in_=g1r_sb, func=AF.Gelu, scale=1.0
        )
        attnv_block(1, 3, h13)
        # fc2 for the gelu0 half fills PE while the residual(1) chain runs
        for tcn in range(0, 4):
            fc2_block(tcn)
        residual_ln2_block(1, batched=False)

        # tail fc2 rides the fused-gelu stream: each jp pair of gelus
        # immediately feeds its 4 accumulation matmuls (psum pairs in the
        # now-free sc banks), so the last gelu only gates one jp round
        pts = [
            psum.tile([P, 1024], fp32, tag="sc", name="fc2_45"),
            psum.tile([P, 1024], fp32, tag="sc", name="fc2_67"),
        ]
        for mh in range(12):
            fc1_group(1, mh)
            if mh % 2 == 1:
                jp = mh // 2
                for tcn in range(4, 8):
                    nc.tensor.matmul(
                        pts[(tcn - 4) // 2][:, (tcn % 2) * 512 : (tcn % 2 + 1) * 512],
                        lhsT=g1T_sb[:, 2 * jp : 2 * jp + 2, tcn * P : (tcn + 1) * P],
                        rhs=w2_sb[:, 2 * jp : 2 * jp + 2, :],
                        start=(jp == 0), stop=(jp == 5), perf_mode=DR,
                        skip_group_check=True,
                    )
        for tcn in range(4, 8):
            ot = work.tile([P, E], fp32, tag="ot", name=f"ot_{tcn}")
            nc.vector.tensor_tensor(
                out=ot,
                in0=pts[(tcn - 4) // 2][:, (tcn % 2) * 512 : (tcn % 2 + 1) * 512],
                in1=xq_sb[:, tcn, :], op=ALU.add,
            )
            eng = {4: nc.scalar, 5: nc.gpsimd, 6: nc.scalar, 7: nc.sync}.get(tcn)
            eng.dma_start(out=out_view[tcn], in_=ot)

    if split_waits:
        _split_matmul_waits(nc, mybir)
    return nc


def _split_matmul_waits(nc, mybir):
    """walrus allows only one sync wait per engine instruction; hoist extra
    waits onto same-engine NoOps placed just before (NX dispatch is in-order,
    so the nops' waits gate the instruction)."""
    k = 0
    for fn in nc.m.functions:
        for blk in fn.blocks:
            new = []
            for inst in blk.instructions:
                si = inst.sync_info
                if si is not None and si.on_wait and len(si.on_wait) > 1:
                    for w in si.on_wait[:-1]:
                        nop = mybir.InstNoOp(name=f"waitnop-{k}", ins=[], outs=[])
                        k += 1
                        nop.engine = inst.engine
                        nop.sync_info = mybir.SyncInfo(on_wait=[w], on_update=[])
                        new.append(nop)
                    inst.sync_info = mybir.SyncInfo(
                        on_wait=[si.on_wait[-1]], on_update=si.on_update
                    )
                new.append(inst)
            blk.instructions[:] = new


def _get_nc():
    if "nc" not in _NC_CACHE:
        _NC_CACHE["nc"] = _build_nc()
    return _NC_CACHE["nc"]


def _qk_slot_perm():
    # perm[j]: source channel (within a heads-major 512-col q or k section)
    # for permuted column j = m*128 + p, m = (quartet x)*2 + slot s; the
    # psum partition p then lands head 4x + p//32, channel 32s + p%32.
    perm = np.empty(512, np.int64)
    for m in range(4):
        x, s = m // 2, m % 2
        for p in range(P):
            perm[m * 128 + p] = (4 * x + p // 32) * 64 + s * 32 + (p % 32)
    return perm


def _prep_inputs(inputs):
    fp8 = ml_dtypes.float8_e4m3
    bf16 = ml_dtypes.bfloat16
    x = np.asarray(inputs["x"], np.float32)
    qkv_w = np.asarray(inputs["qkv_w"], np.float32)
    qkv_b = np.asarray(inputs["qkv_b"], np.float32)
    fc1_w = np.asarray(inputs["fc1_w"], np.float32)
    fc1_b = np.asarray(inputs["fc1_b"], np.float32)
    fc2_w = np.asarray(inputs["fc2_w"], np.float32)
    fc2_b = np.asarray(inputs["fc2_b"], np.float32)

    # reorder qkv channels: per-head interleave [q|k|v]*H -> heads-major
    # [Q|K|V], then slot-permute Q and K columns for the DoubleRow layout
    w3 = qkv_w.reshape(E, H, 3, HD)
    wq, wk, wv = (w3[:, :, i, :].reshape(E, E) for i in range(3))
    b3 = qkv_b.reshape(H, 3, HD)
    bq, bk, bv = (np.ascontiguousarray(b3[:, i, :].reshape(E)) for i in range(3))

    perm = _qk_slot_perm()
    wqkv_f = np.ascontiguousarray(
        np.concatenate([wq[:, perm], wk[:, perm], wv], axis=1)
    )
    wqkv = wqkv_f.astype(bf16)
    wqkv8 = wqkv_f.astype(fp8)
    bq_t = np.ascontiguousarray(bq[perm].reshape(4, P).T)  # [P, 4]
    bk_t = np.ascontiguousarray(bk[perm].reshape(4, P).T)

    w1 = np.ascontiguousarray(fc1_w).astype(bf16)
    w2 = np.ascontiguousarray(fc2_w).astype(fp8)

    in_maps = []
    for c in range(NCORES):
        b, half = c // 2, c % 2
        xr = np.ascontiguousarray(np.roll(x[b], -half * NQ, axis=0))
        in_maps.append(
            {
                "x": xr,
                "wqkv": wqkv,
                "wqkv8": wqkv8,
                "bq": bq_t,
                "bk": bk_t,
                "bv": bv,
                "w1": w1,
                "b1": fc1_b,
                "w2": w2,
                "b2": fc2_b,
            }
        )
    return in_maps


def kernel(**inputs) -> np.ndarray:
    from concourse.bass_utils import run_bass_kernel_spmd

    nc = _get_nc()
    in_maps = _prep_inputs(inputs)
    res = run_bass_kernel_spmd(nc, in_maps, core_ids=list(range(NCORES)))
    y = np.empty((B, N, E), np.float32)
    for c in range(NCORES):
        b, half = c // 2, c % 2
        y[b, half * NQ : (half + 1) * NQ] = np.asarray(res.results[c]["out"])
    return y


if __name__ == "__main__":
    nc = _build_nc()
    print("build OK")
